# revision 11
# baseline (speedup 1.0000x reference)
"""Deformable-DETR transformer encoder layer on 8 Trainium2 NeuronCores.

Sharding: data-parallel over batch (B=2 -> 4 cores per batch element),
sequence-parallel over queries within the batch group.

Value memory layout: per (head, level) the x-pair rows [v(y,x)|v(y,x+1)]
are stored COLUMN-major (row index = (x+1)*(H+2) + (y+1)), so rows j and
j+1 hold all 4 bilinear corners of one sample point. One dma_gather index
per point (elem_size=128 f32, elem_step=64 overlapping rows) halves the
SWDGE descriptor-generation cost vs. a per-corner-pair gather.

Self-contained: hardcodes all shapes/constants from the problem spec.
"""

import numpy as np

import concourse.bass as bass
import concourse.mybir as mybir
import concourse.tile as tile
from concourse import bacc
from concourse.bass_utils import run_bass_kernel_spmd

F32 = mybir.dt.float32
I32 = mybir.dt.int32
I16 = mybir.dt.int16
BF16 = mybir.dt.bfloat16

# ---- problem constants -------------------------------------------------
SPATIAL = [(100, 100), (50, 50), (25, 25), (13, 13)]
LEVEL_START = [0, 10000, 12500, 13125]
LEN = 13294
D = 256
NH = 8
NL = 4
NP = 4
DH = 32
DFF = 1024
EPS = 1e-5

PAD_LEN = 13312           # 104 * 128, full-sequence padded length
N_FULL_TILES = PAD_LEN // 128
Q_SH = 3328               # 26 * 128, per-core query shard (padded)
N_Q_TILES = Q_SH // 128

# column-major x-pair value table geometry (per head)
RSPC = [h + 2 for h, w in SPATIAL]        # rows per column = H+2
TCOLS = [w + 1 for h, w in SPATIAL]       # columns = W+1 (x in [-1, W-1])
LRB = [0]
for l in range(NL):
    LRB.append(LRB[-1] + TCOLS[l] * RSPC[l])
RPH = LRB[-1]                              # rows per head = 13866
LRB = LRB[:-1]
TBL_ROWS = NH * RPH + 2                    # +2 pad rows for tail reads
TBL_ELEMS = TBL_ROWS * 64

NT = NH * NL * NP         # 128 (h,l,p) triples
GIDX = NT * 128           # idx per tile = 128 q * 128 points (4 calls x 4096)

TWO23 = float(3 << 22)  # 1.5*2^23 magic round constant


def _ap(t, offset_elems, dims):
    """Custom free-dim AP view of an SBUF tile (keeps full 128 partitions)."""
    base = t[:]
    return bass.AP(base.tensor, base.offset + offset_elems, [list(base.ap[0])] + [list(d) for d in dims])


def build(dbg=False, ablate=()):
    nc = bacc.Bacc("TRN2", target_bir_lowering=False, debug=False, num_devices=8)
    A = mybir.AluOpType
    ACTF = mybir.ActivationFunctionType

    def param(name, shape, dtype=F32, out=False):
        return nc.declare_dram_parameter(name, list(shape), dtype, isOutput=out)

    src_full = param("src_full", [PAD_LEN, D])
    srcq = param("srcq", [Q_SH, D])
    posq = param("posq", [Q_SH, D])
    refq = param("refq", [Q_SH, NL * 2])
    Wv = param("Wv", [D, D], BF16)
    Woff = param("Woff", [D, D])
    Wattn = param("Wattn", [D, NT])
    Wout = param("Wout", [D, D])
    W1 = param("W1", [D, DFF], BF16)
    W2 = param("W2", [DFF, D], BF16)
    bv = param("bv", [1, D])
    boff = param("boff", [1, D])
    battn = param("battn", [1, NT])
    bout = param("bout", [1, D])
    b1 = param("b1", [1, DFF])
    b2 = param("b2", [1, D])
    g1r = param("g1r", [128, D])
    be1r = param("be1r", [128, D])
    g2r = param("g2r", [128, D])
    be2r = param("be2r", [128, D])
    ident = param("ident", [128, 128])
    ones_row = param("ones_row", [1, 128])
    cW = param("cW", [128, NT])
    cH = param("cH", [128, NT])
    cWm1 = param("cWm1", [128, NT])
    cHm1 = param("cHm1", [128, NT])
    cWm2 = param("cWm2", [128, NT])
    cHm2 = param("cHm2", [128, NT])
    cRSPC = param("cRSPC", [128, NT])
    cB2 = param("cB2", [128, NT])
    dims8 = param("dims8", [128, NL * 2])
    Sall = param("Sall", [128, 8 * 128])
    zeros8k = param("zeros8k", [1, 8192])
    outq = param("outq", [Q_SH, D], out=True)
    if dbg:
        d_ofs = param("d_ofs", [Q_SH, NT], out=True)
        d_aw = param("d_aw", [Q_SH, NT], out=True)
        d_w4 = param("d_w4", [Q_SH, 4 * NT], out=True)
        d_samp = param("d_samp", [Q_SH, D], out=True)
        d_x0 = param("d_x0", [Q_SH, NT], out=True)
        d_y0 = param("d_y0", [Q_SH, NT], out=True)

    with tile.TileContext(nc) as tc:
        with (
            tc.tile_pool(name="const", bufs=1) as cp,
            tc.tile_pool(name="dram", bufs=1, space="DRAM") as dp,
        ):
            valN = dp.tile([PAD_LEN, D], F32, tag="valN")
            value_t = dp.tile([TBL_ROWS, 64], F32, tag="value")

            def cload(src_ap, p, n, tag):
                t = cp.tile([p, n], F32, tag=tag)
                nc.sync.dma_start(t[:], src_ap[:])
                return t

            tWv = cp.tile([128, 2 * D], BF16, tag="tWv")
            nc.sync.dma_start(tWv[:, 0:D], Wv[0:128, :])
            nc.sync.dma_start(tWv[:, D:2 * D], Wv[128:256, :])
            tid = cload(ident, 128, 128, "tid")
            tbv = cload(bv, 1, D, "tbv")
            tones = cload(ones_row, 1, 128, "tones")
            tidb = cp.tile([128, 128], BF16, tag="tidb")
            nc.vector.tensor_copy(out=tidb[:], in_=tid[:])
            tonesb = cp.tile([1, 128], BF16, tag="tonesb")
            nc.vector.tensor_copy(out=tonesb[:], in_=tones[:])
            tbvb = cp.tile([1, D], BF16, tag="tbvb")
            nc.vector.tensor_copy(out=tbvb[:], in_=tbv[:])
            tWoff = cp.tile([128, 2 * D], F32, tag="tWoff")
            nc.sync.dma_start(tWoff[:, 0:D], Woff[0:128, :])
            nc.sync.dma_start(tWoff[:, D:2 * D], Woff[128:256, :])
            tWattn = cp.tile([128, 2 * NT], F32, tag="tWattn")
            nc.sync.dma_start(tWattn[:, 0:NT], Wattn[0:128, :])
            nc.sync.dma_start(tWattn[:, NT:2 * NT], Wattn[128:256, :])
            tWout = cp.tile([128, 2 * D], F32, tag="tWout")
            nc.sync.dma_start(tWout[:, 0:D], Wout[0:128, :])
            nc.sync.dma_start(tWout[:, D:2 * D], Wout[128:256, :])
            tW1 = cp.tile([128, 2 * DFF], BF16, tag="tW1")
            nc.sync.dma_start(tW1[:, 0:DFF], W1[0:128, :])
            nc.sync.dma_start(tW1[:, DFF:2 * DFF], W1[128:256, :])
            tW2 = cp.tile([128, 8 * D], BF16, tag="tW2")
            for j in range(8):
                nc.sync.dma_start(tW2[:, j * D:(j + 1) * D], W2[j * 128:(j + 1) * 128, :])

            tboff = cload(boff, 1, D, "tboff")
            tbattn = cload(battn, 1, NT, "tbattn")
            tbout = cload(bout, 1, D, "tbout")
            tb1 = cload(b1, 1, DFF, "tb1")
            tb2 = cload(b2, 1, D, "tb2")
            tg1 = cload(g1r, 128, D, "tg1")
            tbe1 = cload(be1r, 128, D, "tbe1")
            tg2 = cload(g2r, 128, D, "tg2")
            tbe2 = cload(be2r, 128, D, "tbe2")
            tcW = cload(cW, 128, NT, "tcW")
            tcH = cload(cH, 128, NT, "tcH")
            tcWm1 = cload(cWm1, 128, NT, "tcWm1")
            tcHm1 = cload(cHm1, 128, NT, "tcHm1")
            tcWm2 = cload(cWm2, 128, NT, "tcWm2")
            tcHm2 = cload(cHm2, 128, NT, "tcHm2")
            tcRSPC = cload(cRSPC, 128, NT, "tcRSPC")
            tcB2 = cload(cB2, 128, NT, "tcB2")
            tdims8 = cload(dims8, 128, NL * 2, "tdims8")
            tSall = cload(Sall, 128, 8 * 128, "tSall")

            # bf16 copies for mixed-precision matmuls
            tb1b = cp.tile([1, DFF], BF16, tag="tb1b")
            nc.vector.tensor_copy(out=tb1b[:], in_=tb1[:])
            tb2b = cp.tile([1, D], BF16, tag="tb2b")
            nc.vector.tensor_copy(out=tb2b[:], in_=tb2[:])

            # small scalar constants for ACT bias operands
            def cconst(val, tag):
                t = cp.tile([128, 1], F32, tag=tag)
                nc.vector.memset(t[:], val)
                return t

            t23 = cconst(TWO23, "t23")
            tm23 = cconst(-TWO23, "tm23")
            tone1 = cconst(1.0, "tone1")
            teps = cconst(EPS, "teps")

            # ---------------- Phase A: value table ---------------------
            # A0: zero the pad regions the c0/c1 passes never write (they
            # are weight-masked but must be finite): row 0 and row H+1 of
            # every column, col 0 els 0:32, col W els 32:64, tail pad rows.
            vb = value_t[:]
            zb = zeros8k[0:1, :]
            zeng = [nc.sync, nc.scalar]
            for l, (H, W) in enumerate(SPATIAL):
                e = zeng[l % 2]
                for r0 in (0, H + 1):
                    e.dma_start(
                        bass.AP(vb.tensor, vb.offset + (LRB[l] + r0) * 64,
                                [[RPH * 64, NH], [RSPC[l] * 64, W + 1], [1, 64]]),
                        bass.AP(zb.tensor, zb.offset,
                                [[0, NH], [0, W + 1], [1, 64]]))
                e.dma_start(
                    bass.AP(vb.tensor, vb.offset + LRB[l] * 64,
                            [[RPH * 64, NH], [64, RSPC[l]], [1, 32]]),
                    bass.AP(zb.tensor, zb.offset, [[0, NH], [0, RSPC[l]], [1, 32]]))
                e.dma_start(
                    bass.AP(vb.tensor, vb.offset + (LRB[l] + W * RSPC[l]) * 64 + 32,
                            [[RPH * 64, NH], [64, RSPC[l]], [1, 32]]),
                    bass.AP(zb.tensor, zb.offset, [[0, NH], [0, RSPC[l]], [1, 32]]))
            nc.sync.dma_start(
                bass.AP(vb.tensor, vb.offset + NH * RPH * 64, [[1, 128]]),
                bass.AP(zb.tensor, zb.offset, [[1, 128]]))

            # A1: natural-layout value projection valN = src @ Wv + bv
            with (
                tc.tile_pool(name="pA", bufs=3) as pA,
                tc.tile_pool(name="psA", bufs=2, space="PSUM") as psA,
                tc.tile_pool(name="psA2", bufs=2, space="PSUM") as psA2,
            ):
                for i in range(0 if "noa" in ablate else N_FULL_TILES):
                    rs = slice(i * 128, (i + 1) * 128)
                    s = pA.tile([128, D], F32, tag="As")
                    nc.sync.dma_start(s[:], src_full[rs, :])
                    sb = pA.tile([128, D], BF16, tag="Asb")
                    nc.vector.tensor_copy(out=sb[:], in_=s[:])
                    sT = pA.tile([128, 2, 128], BF16, tag="AsT")
                    for k in range(2):
                        tp = psA.tile([128, 128], BF16, tag="Atp")
                        nc.tensor.transpose(tp[:], sb[:, k * 128:(k + 1) * 128], tidb[:])
                        nc.vector.tensor_copy(out=sT[:, k, :], in_=tp[:])
                    vp = psA2.tile([128, D], F32, tag="Avp")
                    nc.tensor.matmul(vp[:], lhsT=sT[:, 0, :], rhs=tWv[:, 0:D], start=True, stop=False)
                    nc.tensor.matmul(vp[:], lhsT=sT[:, 1, :], rhs=tWv[:, D:2 * D], start=False, stop=False)
                    nc.tensor.matmul(vp[:], lhsT=tonesb[:], rhs=tbvb[:], start=False, stop=True)
                    vo = pA.tile([128, D], F32, tag="Avo")
                    nc.scalar.copy(vo[:], vp[:])
                    nc.gpsimd.dma_start(valN[rs, :], vo[:])

            # A2: DRAM->DRAM restructuring into the column-major x-pair
            # table. c0 pass: v(y,x) -> col x+1 els 0:32; c1 pass:
            # v(y,x) -> col x els 32:64.
            vnb = valN[:]
            if "noa" not in ablate:
                for h in range(NH):
                    for l, (H, W) in enumerate(SPATIAL):
                        src = bass.AP(
                            vnb.tensor, vnb.offset + LEVEL_START[l] * D + h * DH,
                            [[D, W], [W * D, H], [1, DH]])
                        dst0 = bass.AP(
                            vb.tensor,
                            vb.offset + (h * RPH + LRB[l] + RSPC[l] + 1) * 64,
                            [[RSPC[l] * 64, W], [64, H], [1, DH]])
                        nc.sync.dma_start(dst0, src)
                        dst1 = bass.AP(
                            vb.tensor,
                            vb.offset + (h * RPH + LRB[l] + 1) * 64 + DH,
                            [[RSPC[l] * 64, W], [64, H], [1, DH]])
                        nc.scalar.dma_start(dst1, src)

            # ---------------- Phase B: per-query-tile -------------------
            with (
                tc.tile_pool(name="pIn", bufs=3) as pIn,
                tc.tile_pool(name="pPr", bufs=2) as pPr,
                tc.tile_pool(name="pW4", bufs=3) as pW4,
                tc.tile_pool(name="pTw", bufs=4) as pTw,
                tc.tile_pool(name="pG", bufs=4) as pG,
                tc.tile_pool(name="pSW", bufs=3) as pSW,
                tc.tile_pool(name="pFin", bufs=2) as pFin,
                tc.tile_pool(name="psT", bufs=1, space="PSUM") as psT,
                tc.tile_pool(name="psMM", bufs=2, space="PSUM") as psMM,
                tc.tile_pool(name="psTw", bufs=2, space="PSUM") as psTw,
                tc.tile_pool(name="psO", bufs=2, space="PSUM") as psO,
                tc.tile_pool(name="psF", bufs=1, space="PSUM") as psF,
            ):

                def prep(i):
                    rs = slice(i * 128, (i + 1) * 128)
                    s = pIn.tile([128, D], F32, tag="Bs")
                    nc.sync.dma_start(s[:], srcq[rs, :])
                    p = pIn.tile([128, D], F32, tag="Bp")
                    nc.sync.dma_start(p[:], posq[rs, :])
                    r8 = pIn.tile([128, NL * 2], F32, tag="Br8")
                    nc.sync.dma_start(r8[:], refq[rs, :])

                    q = pPr.tile([128, D], F32, tag="Bq")
                    nc.vector.tensor_tensor(out=q[:], in0=s[:], in1=p[:], op=A.add)
                    qT = pPr.tile([128, 2, 128], F32, tag="BqT")
                    for k in range(2):
                        tp = psT.tile([128, 128], F32, tag="Btp")
                        nc.tensor.transpose(tp[:], q[:, k * 128:(k + 1) * 128], tid[:])
                        nc.scalar.copy(qT[:, k, :], tp[:])

                    qmm = psMM.tile([128, D + NT], F32, tag="Bqmm")
                    offp = qmm[:, 0:D]
                    nc.tensor.matmul(offp, lhsT=qT[:, 0, :], rhs=tWoff[:, 0:D], start=True, stop=False)
                    nc.tensor.matmul(offp, lhsT=qT[:, 1, :], rhs=tWoff[:, D:2 * D], start=False, stop=False)
                    nc.tensor.matmul(offp, lhsT=tones[:], rhs=tboff[:], start=False, stop=True)

                    attp = qmm[:, D:D + NT]
                    nc.tensor.matmul(attp, lhsT=qT[:, 0, :], rhs=tWattn[:, 0:NT], start=True, stop=False)
                    nc.tensor.matmul(attp, lhsT=qT[:, 1, :], rhs=tWattn[:, NT:2 * NT], start=False, stop=False)
                    nc.tensor.matmul(attp, lhsT=tones[:], rhs=tbattn[:], start=False, stop=True)

                    # softmax over the 16 (l,p) per head
                    mx = pPr.tile([128, NH], F32, tag="Bmx")
                    nc.vector.tensor_reduce(
                        out=mx[:], in_=_ap(qmm, D, [[16, NH], [1, 16]]),
                        axis=mybir.AxisListType.X, op=A.max)
                    xs = pPr.tile([128, NT], F32, tag="Bxs")
                    nc.vector.tensor_tensor(
                        out=xs[:], in0=attp,
                        in1=_ap(mx, 0, [[1, NH], [0, 16]]), op=A.subtract)
                    es = pPr.tile([128, NT], F32, tag="Bes")
                    nc.scalar.activation(es[:], xs[:], ACTF.Exp)
                    sm = pPr.tile([128, NH], F32, tag="Bsm")
                    nc.vector.tensor_reduce(
                        out=sm[:], in_=_ap(es, 0, [[16, NH], [1, 16]]),
                        axis=mybir.AxisListType.X, op=A.add)
                    rcp = pPr.tile([128, NH], F32, tag="Brcp")
                    nc.vector.reciprocal(rcp[:], sm[:])
                    aw = pPr.tile([128, NT], F32, tag="Baw")
                    nc.vector.tensor_tensor(
                        out=aw[:], in0=es[:],
                        in1=_ap(rcp, 0, [[1, NH], [0, 16]]), op=A.mult)

                    # sampling positions: px = (off - 0.5) + (ref*WH) broadcast
                    rsc = pPr.tile([128, NL * 2], F32, tag="Brsc")
                    nc.vector.tensor_tensor(out=rsc[:], in0=r8[:], in1=tdims8[:], op=A.mult)
                    r32 = pPr.tile([128, 32], F32, tag="Br32")
                    nc.vector.tensor_copy(out=r32[:], in_=_ap(rsc, 0, [[2, NL], [0, NP], [1, 2]]))
                    px = pPr.tile([128, D], F32, tag="Bpx")
                    nc.vector.scalar_tensor_tensor(
                        out=px[:], in0=offp, scalar=-0.5,
                        in1=_ap(r32, 0, [[0, NH], [1, 32]]), op0=A.add, op1=A.add)

                    # clip to [-1, dim]
                    xt = pPr.tile([128, NT], F32, tag="Bxt")
                    nc.vector.scalar_tensor_tensor(
                        out=xt[:], in0=_ap(px, 0, [[2, NT]]), scalar=-1.0,
                        in1=tcW[:], op0=A.max, op1=A.min)
                    yt = pPr.tile([128, NT], F32, tag="Byt")
                    nc.vector.scalar_tensor_tensor(
                        out=yt[:], in0=_ap(px, 1, [[2, NT]]), scalar=-1.0,
                        in1=tcH[:], op0=A.max, op1=A.min)

                    # floor + frac (round-to-int via 2^23 trick, then fix up)
                    def floor_frac(src, tagp):
                        r2 = pPr.tile([128, NT], F32, tag=tagp + "r2")
                        nc.scalar.activation(r2[:], src[:], ACTF.Identity, bias=t23[:, 0:1])
                        rn = pPr.tile([128, NT], F32, tag=tagp + "rn")
                        nc.scalar.activation(rn[:], r2[:], ACTF.Identity, bias=tm23[:, 0:1])
                        fx = pPr.tile([128, NT], F32, tag=tagp + "fx")
                        nc.vector.tensor_tensor(out=fx[:], in0=rn[:], in1=src[:], op=A.is_gt)
                        fl = pPr.tile([128, NT], F32, tag=tagp + "fl")
                        nc.vector.tensor_tensor(out=fl[:], in0=rn[:], in1=fx[:], op=A.subtract)
                        fr = pPr.tile([128, NT], F32, tag=tagp + "fr")
                        nc.vector.tensor_tensor(out=fr[:], in0=src[:], in1=fl[:], op=A.subtract)
                        return fl, fr

                    x0, dx = floor_frac(xt, "Bx")
                    y0, dy = floor_frac(yt, "By")

                    # corner weights with zero-padding masks
                    def corner_w(f0, dfrac, cM1, cM2, tagp):
                        inb1 = pPr.tile([128, NT], F32, tag=tagp + "i1")
                        nc.vector.tensor_tensor(out=inb1[:], in0=f0[:], in1=cM1[:], op=A.is_le)
                        m0 = pPr.tile([128, NT], F32, tag=tagp + "m0")
                        nc.vector.scalar_tensor_tensor(
                            out=m0[:], in0=f0[:], scalar=0.0, in1=inb1[:],
                            op0=A.is_ge, op1=A.mult)
                        m1 = pPr.tile([128, NT], F32, tag=tagp + "m1")
                        nc.vector.tensor_tensor(out=m1[:], in0=f0[:], in1=cM2[:], op=A.is_le)
                        om = pPr.tile([128, NT], F32, tag=tagp + "om")
                        nc.scalar.activation(om[:], dfrac[:], ACTF.Identity, bias=tone1[:, 0:1], scale=-1.0)
                        w0 = pPr.tile([128, NT], F32, tag=tagp + "w0")
                        nc.vector.tensor_tensor(out=w0[:], in0=om[:], in1=m0[:], op=A.mult)
                        w1 = pPr.tile([128, NT], F32, tag=tagp + "w1")
                        nc.vector.tensor_tensor(out=w1[:], in0=dfrac[:], in1=m1[:], op=A.mult)
                        return w0, w1

                    wx0, wx1 = corner_w(x0, dx, tcWm1, tcWm2, "BX")
                    wy0, wy1 = corner_w(y0, dy, tcHm1, tcHm2, "BY")

                    wy0a = pPr.tile([128, NT], F32, tag="Bwy0a")
                    nc.vector.tensor_tensor(out=wy0a[:], in0=wy0[:], in1=aw[:], op=A.mult)
                    wy1a = pPr.tile([128, NT], F32, tag="Bwy1a")
                    nc.vector.tensor_tensor(out=wy1a[:], in0=wy1[:], in1=aw[:], op=A.mult)

                    w4 = pW4.tile([128, 4 * NT], F32, tag="Bw4")
                    for jj, (wyj, wxk) in enumerate(
                        [(wy0a, wx0), (wy0a, wx1), (wy1a, wx0), (wy1a, wx1)]
                    ):
                        nc.vector.tensor_tensor(
                            out=_ap(w4, jj, [[4, NT]]), in0=wyj[:], in1=wxk[:], op=A.mult)

                    # gather row index: (x0c+1)*RSPC + (y0a+1) + head/level base
                    x0c = pPr.tile([128, NT], F32, tag="Bx0c")
                    nc.vector.tensor_tensor(out=x0c[:], in0=x0[:], in1=tcWm1[:], op=A.min)
                    y0a = pPr.tile([128, NT], F32, tag="By0a")
                    nc.vector.tensor_tensor(out=y0a[:], in0=y0[:], in1=tcHm1[:], op=A.min)
                    of1 = pPr.tile([128, NT], F32, tag="Bof1")
                    nc.vector.tensor_tensor(out=of1[:], in0=x0c[:], in1=tcRSPC[:], op=A.mult)
                    of2 = pPr.tile([128, NT], F32, tag="Bof2")
                    nc.vector.tensor_tensor(out=of2[:], in0=of1[:], in1=y0a[:], op=A.add)
                    offs = pPr.tile([128, NT], F32, tag="Boffs")
                    nc.vector.tensor_tensor(out=offs[:], in0=of2[:], in1=tcB2[:], op=A.add)

                    # wrapped idx tile: Tw[p, t*256 + j*8 + qh] = offs(16qh+p%16, t*32+j)
                    Tw = pTw.tile([128, 4 * 256], I16, tag="BTw")
                    Twb = Tw[:]
                    for qh in range(8):
                        po = psTw.tile([128, 128], F32, tag="Bpo")
                        nc.tensor.matmul(po[:], lhsT=tSall[:, qh * 128:(qh + 1) * 128],
                                         rhs=offs[:], start=True, stop=True)
                        nc.scalar.copy(
                            bass.AP(Twb.tensor, Twb.offset + qh,
                                    [list(Twb.ap[0]), [256, 4], [8, 32]]),
                            po[:])
                    return (rs, s, w4, Tw, offs if dbg else None, aw if dbg else None,
                            x0 if dbg else None, y0 if dbg else None)

                def sample(st):
                    rs, s, w4, Tw, d_offs_t, d_aw_t, d_x0_t, d_y0_t = st
                    vtb = value_t[:]
                    samp = pFin.tile([128, D], F32, tag="Bsamp")
                    for t in range(4):
                        g = pG.tile([128, 32, 128], F32, tag="Bg")
                        if "nogather" in ablate:
                            nc.vector.memset(g[:, 0, :], 0.0)
                        else:
                            nc.gpsimd.dma_gather(
                                out_ap=g[:],
                                in_ap=bass.AP(vtb.tensor, vtb.offset + t * 2 * RPH * 64,
                                              [[64, 2 * RPH], [1, 128]]),
                                idxs_ap=Tw[:, t * 256:(t + 1) * 256], num_idxs=4096,
                                num_idxs_reg=4096, elem_size=128, elem_step=64,
                                single_packet=False)
                        if "nosamp" in ablate:
                            nc.vector.memset(samp[:, t * 64:(t + 1) * 64], 0.0)
                            continue
                        QB = 4096
                        sw = pSW.tile([128, QB], BF16, tag="Bsw")
                        nc.vector.tensor_tensor(
                            out=_ap(sw, 0, [[32, 128], [1, 32]]),
                            in0=_ap(g, 0, [[32, 128], [1, 32]]),
                            in1=_ap(w4, t * 128, [[1, 128], [0, 32]]),
                            op=A.mult)
                        # in-place pairwise tree over the 128 32-el blocks
                        for n in (64, 32, 16, 8, 4):
                            nc.vector.tensor_tensor(
                                out=_ap(sw, 0, [[32, n], [1, 32]]),
                                in0=_ap(sw, 0, [[64, n], [1, 32]]),
                                in1=_ap(sw, 32, [[64, n], [1, 32]]), op=A.add)
                        nc.vector.tensor_tensor(
                            out=samp[:, t * 64:(t + 1) * 64],
                            in0=_ap(sw, 0, [[64, 2], [1, 32]]),
                            in1=_ap(sw, 32, [[64, 2], [1, 32]]), op=A.add)

                    # output projection
                    sT = pFin.tile([128, 2, 128], F32, tag="BsT")
                    for k in range(2):
                        tp = psT.tile([128, 128], F32, tag="Btp")
                        nc.tensor.transpose(tp[:], samp[:, k * 128:(k + 1) * 128], tid[:])
                        nc.scalar.copy(sT[:, k, :], tp[:])
                    o2p = psO.tile([128, D], F32, tag="Bo23")
                    nc.tensor.matmul(o2p[:], lhsT=sT[:, 0, :], rhs=tWout[:, 0:D], start=True, stop=False)
                    nc.tensor.matmul(o2p[:], lhsT=sT[:, 1, :], rhs=tWout[:, D:2 * D], start=False, stop=False)
                    nc.tensor.matmul(o2p[:], lhsT=tones[:], rhs=tbout[:], start=False, stop=True)

                    # residual + layernorm
                    def layer_norm(inp_sbuf, res_psum, gt, bt, tagp):
                        x1 = pFin.tile([128, D], F32, tag="BLx1")
                        sums = pFin.tile([128, 1], F32, tag="BLsu")
                        nc.vector.scalar_tensor_tensor(
                            out=x1[:], in0=inp_sbuf[:], scalar=0.0, in1=res_psum[:],
                            op0=A.add, op1=A.add, accum_out=sums[:])
                        negm = pFin.tile([128, 1], F32, tag="BLnm")
                        nc.scalar.mul(negm[:], sums[:], -1.0 / D)
                        sq = pFin.tile([128, D], F32, tag="BLsq")
                        ssq = pFin.tile([128, 1], F32, tag="BLss")
                        nc.scalar.activation(sq[:], x1[:], ACTF.Square,
                                             bias=negm[:, 0:1], accum_out=ssq[:])
                        sd = pFin.tile([128, 1], F32, tag="BLsd")
                        nc.scalar.activation(sd[:], ssq[:], ACTF.Sqrt,
                                             scale=1.0 / D, bias=teps[:, 0:1])
                        rstd = pFin.tile([128, 1], F32, tag="BLrs")
                        nc.vector.reciprocal(rstd[:], sd[:])
                        xh = pFin.tile([128, D], F32, tag="BLxh")
                        nc.vector.tensor_scalar(
                            out=xh[:], in0=x1[:], scalar1=negm[:, 0:1],
                            scalar2=rstd[:, 0:1], op0=A.add, op1=A.mult)
                        yv = pFin.tile([128, D], F32, tag=tagp + "y")
                        nc.vector.tensor_tensor(out=yv[:], in0=xh[:], in1=gt[:], op=A.mult)
                        nc.vector.tensor_tensor(out=yv[:], in0=yv[:], in1=bt[:], op=A.add)
                        return yv

                    y1v = layer_norm(s, o2p, tg1, tbe1, "BL1")

                    # FFN
                    yT = pFin.tile([128, 2, 128], BF16, tag="ByT")
                    for k in range(2):
                        tp = psT.tile([128, 128], F32, tag="Btp")
                        nc.tensor.transpose(tp[:], y1v[:, k * 128:(k + 1) * 128], tid[:])
                        nc.scalar.copy(yT[:, k, :], tp[:])
                    h1 = pFin.tile([128, DFF], BF16, tag="Bh1")
                    for j in range(8):
                        js = slice(j * 128, (j + 1) * 128)
                        hp = psF.tile([128, 128], F32, tag="Bhp")
                        nc.tensor.matmul(hp[:], lhsT=tW1[:, 0 * DFF + j * 128:0 * DFF + (j + 1) * 128],
                                         rhs=yT[:, 0, :], start=True, stop=False)
                        nc.tensor.matmul(hp[:], lhsT=tW1[:, 1 * DFF + j * 128:1 * DFF + (j + 1) * 128],
                                         rhs=yT[:, 1, :], start=False, stop=False)
                        nc.tensor.matmul(hp[:], lhsT=tb1b[:, js], rhs=tonesb[:], start=False, stop=True)
                        nc.scalar.activation(h1[:, js], hp[:], ACTF.Relu)
                    o3p = psO.tile([128, D], F32, tag="Bo23")
                    for j in range(8):
                        js = slice(j * 128, (j + 1) * 128)
                        nc.tensor.matmul(o3p[:], lhsT=h1[:, js], rhs=tW2[:, j * D:(j + 1) * D],
                                         start=(j == 0), stop=False)
                    nc.tensor.matmul(o3p[:], lhsT=tonesb[:], rhs=tb2b[:], start=False, stop=True)

                    y2v = layer_norm(y1v, o3p, tg2, tbe2, "BL2")
                    nc.sync.dma_start(outq[rs, :], y2v[:])
                    if dbg:
                        nc.sync.dma_start(d_ofs[rs, :], d_offs_t[:])
                        nc.sync.dma_start(d_aw[rs, :], d_aw_t[:])
                        nc.sync.dma_start(d_w4[rs, :], w4[:])
                        nc.sync.dma_start(d_samp[rs, :], samp[:])
                        nc.sync.dma_start(d_x0[rs, :], d_x0_t[:])
                        nc.sync.dma_start(d_y0[rs, :], d_y0_t[:])

                # 2-stage software pipeline: prep(i+1) issues before
                # sample(i) so every engine's FIFO keeps the gather
                # stream fed.
                ntiles = 0 if "nob" in ablate else N_Q_TILES
                state = None
                for i in range(ntiles + 1):
                    new = prep(i) if i < ntiles else None
                    if state is not None:
                        sample(state)
                    state = new

    nc.compile()
    return nc


# ----------------------------------------------------------------------
# host-side wrapper
# ----------------------------------------------------------------------
_NC_CACHE = None


def _get_nc():
    global _NC_CACHE
    if _NC_CACHE is None:
        _NC_CACHE = build()
    return _NC_CACHE


def make_consts():
    h_i, l_i, p_i = np.meshgrid(np.arange(NH), np.arange(NL), np.arange(NP), indexing="ij")
    Wl = np.array([w for (_, w) in SPATIAL], np.float32)
    Hl = np.array([h for (h, _) in SPATIAL], np.float32)
    lw = Wl[l_i].reshape(-1)
    lh = Hl[l_i].reshape(-1)
    rspc = np.array(RSPC, np.float32)[l_i].reshape(-1)
    lrb = np.array(LRB, np.float32)[l_i].reshape(-1)
    b2 = ((h_i % 2) * RPH).reshape(-1) + lrb + rspc + 1.0
    rep = lambda v: np.tile(v[None, :].astype(np.float32), (128, 1))
    dims8 = np.zeros(NL * 2, np.float32)
    dims8[0::2] = Wl
    dims8[1::2] = Hl
    Sall = np.zeros((128, 8 * 128), np.float32)
    for qh in range(8):
        for q16 in range(16):
            for k in range(8):
                Sall[16 * qh + q16, qh * 128 + 16 * k + q16] = 1.0
    return {
        "cW": rep(lw), "cH": rep(lh),
        "cWm1": rep(lw - 1), "cHm1": rep(lh - 1),
        "cWm2": rep(lw - 2), "cHm2": rep(lh - 2),
        "cRSPC": rep(rspc), "cB2": rep(b2),
        "dims8": rep(dims8),
        "ident": np.eye(128, dtype=np.float32),
        "ones_row": np.ones((1, 128), np.float32),
        "Sall": Sall,
        "zeros8k": np.zeros((1, 8192), np.float32),
    }


SHARD_STARTS = [0, 3324, 6648, 9972]
SHARD_SIZES = [3324, 3324, 3324, 3322]


def make_in_maps(inputs):
    consts = make_consts()
    in_maps = []
    for core in range(8):
        b, c = core // 4, core % 4
        st, sz = SHARD_STARTS[c], SHARD_SIZES[c]
        src_full = np.zeros((PAD_LEN, D), np.float32)
        src_full[:LEN] = inputs["src"][b]
        srcq = np.zeros((Q_SH, D), np.float32)
        srcq[:sz] = inputs["src"][b, st:st + sz]
        posq = np.zeros((Q_SH, D), np.float32)
        posq[:sz] = inputs["pos"][b, st:st + sz]
        refq = np.full((Q_SH, NL * 2), 0.5, np.float32)
        refq[:sz] = inputs["reference_points"][b, st:st + sz].reshape(sz, NL * 2)
        m = {
            "src_full": src_full, "srcq": srcq, "posq": posq, "refq": refq,
            "Wv": inputs["W_value"], "Woff": inputs["W_off"],
            "Wattn": inputs["W_attn"], "Wout": inputs["W_out"],
            "W1": inputs["W1"], "W2": inputs["W2"],
            "bv": inputs["b_value"][None, :], "boff": inputs["b_off"][None, :],
            "battn": inputs["b_attn"][None, :], "bout": inputs["b_out"][None, :],
            "b1": inputs["b1"][None, :], "b2": inputs["b2"][None, :],
            "g1r": np.tile(inputs["g1"][None, :], (128, 1)),
            "be1r": np.tile(inputs["be1"][None, :], (128, 1)),
            "g2r": np.tile(inputs["g2"][None, :], (128, 1)),
            "be2r": np.tile(inputs["be2"][None, :], (128, 1)),
        }
        for k in ("cW", "cH", "cWm1", "cHm1", "cWm2", "cHm2", "cRSPC", "cB2",
                  "dims8", "ident", "ones_row", "Sall", "zeros8k"):
            m[k] = consts[k]
        import ml_dtypes
        bf16_params = {"Wv", "W1", "W2"}
        in_maps.append({
            k: np.ascontiguousarray(v, ml_dtypes.bfloat16 if k in bf16_params else np.float32)
            for k, v in m.items()})
    return in_maps


def assemble_out(results):
    out = np.empty((2, LEN, D), np.float32)
    for core in range(8):
        b, c = core // 4, core % 4
        st, sz = SHARD_STARTS[c], SHARD_SIZES[c]
        out[b, st:st + sz] = results[core]["outq"][:sz]
    return out


def run(inputs, trace=False, **kw):
    nc = _get_nc()
    in_maps = make_in_maps(inputs)
    res = run_bass_kernel_spmd(nc, in_maps, core_ids=list(range(8)), trace=trace, **kw)
    return assemble_out(res.results), res


def kernel(**inputs):
    out, _ = run(inputs)
    return out


# revision 12
# speedup vs baseline: 1.0280x; 1.0280x over previous
"""Deformable-DETR transformer encoder layer on 8 Trainium2 NeuronCores.

Sharding: data-parallel over batch (B=2 -> 4 cores per batch element),
sequence-parallel over queries within the batch group.

Value memory layout: per (head, level) the x-pair rows [v(y,x)|v(y,x+1)]
are stored COLUMN-major (row index = (x+1)*(H+2) + (y+1)), so rows j and
j+1 hold all 4 bilinear corners of one sample point. One dma_gather index
per point (elem_size=128 f32, elem_step=64 overlapping rows) halves the
SWDGE descriptor-generation cost vs. a per-corner-pair gather.

Self-contained: hardcodes all shapes/constants from the problem spec.
"""

import numpy as np

import concourse.bass as bass
import concourse.mybir as mybir
import concourse.tile as tile
from concourse import bacc
from concourse.bass_utils import run_bass_kernel_spmd

F32 = mybir.dt.float32
I32 = mybir.dt.int32
I16 = mybir.dt.int16
BF16 = mybir.dt.bfloat16

# ---- problem constants -------------------------------------------------
SPATIAL = [(100, 100), (50, 50), (25, 25), (13, 13)]
LEVEL_START = [0, 10000, 12500, 13125]
LEN = 13294
D = 256
NH = 8
NL = 4
NP = 4
DH = 32
DFF = 1024
EPS = 1e-5

PAD_LEN = 13312           # 104 * 128, full-sequence padded length
N_FULL_TILES = PAD_LEN // 128
Q_SH = 3328               # 26 * 128, per-core query shard (padded)
N_Q_TILES = Q_SH // 128

# column-major x-pair value table geometry (per head)
RSPC = [h + 2 for h, w in SPATIAL]        # rows per column = H+2
TCOLS = [w + 1 for h, w in SPATIAL]       # columns = W+1 (x in [-1, W-1])
LRB = [0]
for l in range(NL):
    LRB.append(LRB[-1] + TCOLS[l] * RSPC[l])
RPH = LRB[-1]                              # rows per head = 13866
LRB = LRB[:-1]
TBL_ROWS = NH * RPH + 2                    # +2 pad rows for tail reads
TBL_ELEMS = TBL_ROWS * 64

NT = NH * NL * NP         # 128 (h,l,p) triples
GIDX = NT * 128           # idx per tile = 128 q * 128 points (4 calls x 4096)

TWO23 = float(3 << 22)  # 1.5*2^23 magic round constant


def _ap(t, offset_elems, dims):
    """Custom free-dim AP view of an SBUF tile (keeps full 128 partitions)."""
    base = t[:]
    return bass.AP(base.tensor, base.offset + offset_elems, [list(base.ap[0])] + [list(d) for d in dims])


def build(dbg=False, ablate=()):
    nc = bacc.Bacc("TRN2", target_bir_lowering=False, debug=False, num_devices=8)
    A = mybir.AluOpType
    ACTF = mybir.ActivationFunctionType

    def param(name, shape, dtype=F32, out=False):
        return nc.declare_dram_parameter(name, list(shape), dtype, isOutput=out)

    src_full = param("src_full", [PAD_LEN, D])
    srcq = param("srcq", [Q_SH, D])
    posq = param("posq", [Q_SH, D])
    refq = param("refq", [Q_SH, NL * 2])
    Wv = param("Wv", [D, D], BF16)
    Woff = param("Woff", [D, D])
    Wattn = param("Wattn", [D, NT])
    Wout = param("Wout", [D, D])
    W1 = param("W1", [D, DFF], BF16)
    W2 = param("W2", [DFF, D], BF16)
    bv = param("bv", [1, D])
    boff = param("boff", [1, D])
    battn = param("battn", [1, NT])
    bout = param("bout", [1, D])
    b1 = param("b1", [1, DFF])
    b2 = param("b2", [1, D])
    g1r = param("g1r", [128, D])
    be1r = param("be1r", [128, D])
    g2r = param("g2r", [128, D])
    be2r = param("be2r", [128, D])
    ident = param("ident", [128, 128])
    ones_row = param("ones_row", [1, 128])
    cW = param("cW", [128, NT])
    cH = param("cH", [128, NT])
    cWm1 = param("cWm1", [128, NT])
    cHm1 = param("cHm1", [128, NT])
    cWm2 = param("cWm2", [128, NT])
    cHm2 = param("cHm2", [128, NT])
    cRSPC = param("cRSPC", [128, NT])
    cB2 = param("cB2", [128, NT])
    dims8 = param("dims8", [128, NL * 2])
    Sall = param("Sall", [128, 8 * 128])
    zeros8k = param("zeros8k", [1, 8192])
    outq = param("outq", [Q_SH, D], out=True)
    if dbg:
        d_ofs = param("d_ofs", [Q_SH, NT], out=True)
        d_aw = param("d_aw", [Q_SH, NT], out=True)
        d_w4 = param("d_w4", [Q_SH, 4 * NT], out=True)
        d_samp = param("d_samp", [Q_SH, D], out=True)
        d_x0 = param("d_x0", [Q_SH, NT], out=True)
        d_y0 = param("d_y0", [Q_SH, NT], out=True)

    with tile.TileContext(nc) as tc:
        with (
            tc.tile_pool(name="const", bufs=1) as cp,
            tc.tile_pool(name="dram", bufs=1, space="DRAM") as dp,
        ):
            valN = dp.tile([PAD_LEN, D], F32, tag="valN")
            value_t = dp.tile([TBL_ROWS, 64], F32, tag="value")

            def cload(src_ap, p, n, tag):
                t = cp.tile([p, n], F32, tag=tag)
                nc.sync.dma_start(t[:], src_ap[:])
                return t

            tWv = cp.tile([128, 2 * D], BF16, tag="tWv")
            nc.sync.dma_start(tWv[:, 0:D], Wv[0:128, :])
            nc.sync.dma_start(tWv[:, D:2 * D], Wv[128:256, :])
            tid = cload(ident, 128, 128, "tid")
            tbv = cload(bv, 1, D, "tbv")
            tones = cload(ones_row, 1, 128, "tones")
            tidb = cp.tile([128, 128], BF16, tag="tidb")
            nc.vector.tensor_copy(out=tidb[:], in_=tid[:])
            tonesb = cp.tile([1, 128], BF16, tag="tonesb")
            nc.vector.tensor_copy(out=tonesb[:], in_=tones[:])
            tbvb = cp.tile([1, D], BF16, tag="tbvb")
            nc.vector.tensor_copy(out=tbvb[:], in_=tbv[:])
            tWoff = cp.tile([128, 2 * D], F32, tag="tWoff")
            nc.sync.dma_start(tWoff[:, 0:D], Woff[0:128, :])
            nc.sync.dma_start(tWoff[:, D:2 * D], Woff[128:256, :])
            tWattn = cp.tile([128, 2 * NT], F32, tag="tWattn")
            nc.sync.dma_start(tWattn[:, 0:NT], Wattn[0:128, :])
            nc.sync.dma_start(tWattn[:, NT:2 * NT], Wattn[128:256, :])
            tWout = cp.tile([128, 2 * D], F32, tag="tWout")
            nc.sync.dma_start(tWout[:, 0:D], Wout[0:128, :])
            nc.sync.dma_start(tWout[:, D:2 * D], Wout[128:256, :])
            tW1 = cp.tile([128, 2 * DFF], BF16, tag="tW1")
            nc.sync.dma_start(tW1[:, 0:DFF], W1[0:128, :])
            nc.sync.dma_start(tW1[:, DFF:2 * DFF], W1[128:256, :])
            tW2 = cp.tile([128, 8 * D], BF16, tag="tW2")
            for j in range(8):
                nc.sync.dma_start(tW2[:, j * D:(j + 1) * D], W2[j * 128:(j + 1) * 128, :])

            tboff = cload(boff, 1, D, "tboff")
            tbattn = cload(battn, 1, NT, "tbattn")
            tbout = cload(bout, 1, D, "tbout")
            tb1 = cload(b1, 1, DFF, "tb1")
            tb2 = cload(b2, 1, D, "tb2")
            tg1 = cload(g1r, 128, D, "tg1")
            tbe1 = cload(be1r, 128, D, "tbe1")
            tg2 = cload(g2r, 128, D, "tg2")
            tbe2 = cload(be2r, 128, D, "tbe2")
            tcW = cload(cW, 128, NT, "tcW")
            tcH = cload(cH, 128, NT, "tcH")
            tcWm1 = cload(cWm1, 128, NT, "tcWm1")
            tcHm1 = cload(cHm1, 128, NT, "tcHm1")
            tcWm2 = cload(cWm2, 128, NT, "tcWm2")
            tcHm2 = cload(cHm2, 128, NT, "tcHm2")
            tcRSPC = cload(cRSPC, 128, NT, "tcRSPC")
            tcB2 = cload(cB2, 128, NT, "tcB2")
            tdims8 = cload(dims8, 128, NL * 2, "tdims8")
            tSall = cload(Sall, 128, 8 * 128, "tSall")

            # bf16 copies for mixed-precision matmuls
            tb1b = cp.tile([1, DFF], BF16, tag="tb1b")
            nc.vector.tensor_copy(out=tb1b[:], in_=tb1[:])
            tb2b = cp.tile([1, D], BF16, tag="tb2b")
            nc.vector.tensor_copy(out=tb2b[:], in_=tb2[:])

            # small scalar constants for ACT bias operands
            def cconst(val, tag):
                t = cp.tile([128, 1], F32, tag=tag)
                nc.vector.memset(t[:], val)
                return t

            t23 = cconst(TWO23, "t23")
            tm23 = cconst(-TWO23, "tm23")
            tone1 = cconst(1.0, "tone1")
            teps = cconst(EPS, "teps")

            # ---------------- Phase A: value table ---------------------
            # A0: zero the pad regions the c0/c1 passes never write (they
            # are weight-masked but must be finite): row 0 and row H+1 of
            # every column, col 0 els 0:32, col W els 32:64, tail pad rows.
            vb = value_t[:]
            zb = zeros8k[0:1, :]
            zeng = [nc.sync, nc.scalar]
            for l, (H, W) in enumerate(SPATIAL):
                e = zeng[l % 2]
                for r0 in (0, H + 1):
                    e.dma_start(
                        bass.AP(vb.tensor, vb.offset + (LRB[l] + r0) * 64,
                                [[RPH * 64, NH], [RSPC[l] * 64, W + 1], [1, 64]]),
                        bass.AP(zb.tensor, zb.offset,
                                [[0, NH], [0, W + 1], [1, 64]]))
                e.dma_start(
                    bass.AP(vb.tensor, vb.offset + LRB[l] * 64,
                            [[RPH * 64, NH], [64, RSPC[l]], [1, 32]]),
                    bass.AP(zb.tensor, zb.offset, [[0, NH], [0, RSPC[l]], [1, 32]]))
                e.dma_start(
                    bass.AP(vb.tensor, vb.offset + (LRB[l] + W * RSPC[l]) * 64 + 32,
                            [[RPH * 64, NH], [64, RSPC[l]], [1, 32]]),
                    bass.AP(zb.tensor, zb.offset, [[0, NH], [0, RSPC[l]], [1, 32]]))
            nc.sync.dma_start(
                bass.AP(vb.tensor, vb.offset + NH * RPH * 64, [[1, 128]]),
                bass.AP(zb.tensor, zb.offset, [[1, 128]]))

            # A1: natural-layout value projection valN = src @ Wv + bv
            with (
                tc.tile_pool(name="pA", bufs=6) as pA,
                tc.tile_pool(name="psA", bufs=2, space="PSUM") as psA,
                tc.tile_pool(name="psA2", bufs=3, space="PSUM") as psA2,
            ):
                for i in range(0 if "noa" in ablate else N_FULL_TILES):
                    rs = slice(i * 128, (i + 1) * 128)
                    s = pA.tile([128, D], F32, tag="As")
                    nc.sync.dma_start(s[:], src_full[rs, :])
                    sb = pA.tile([128, D], BF16, tag="Asb")
                    nc.vector.tensor_copy(out=sb[:], in_=s[:])
                    sT = pA.tile([128, 2, 128], BF16, tag="AsT")
                    for k in range(2):
                        tp = psA.tile([128, 128], BF16, tag="Atp")
                        nc.tensor.transpose(tp[:], sb[:, k * 128:(k + 1) * 128], tidb[:])
                        nc.vector.tensor_copy(out=sT[:, k, :], in_=tp[:])
                    vp = psA2.tile([128, D], F32, tag="Avp")
                    nc.tensor.matmul(vp[:], lhsT=sT[:, 0, :], rhs=tWv[:, 0:D], start=True, stop=False)
                    nc.tensor.matmul(vp[:], lhsT=sT[:, 1, :], rhs=tWv[:, D:2 * D], start=False, stop=False)
                    nc.tensor.matmul(vp[:], lhsT=tonesb[:], rhs=tbvb[:], start=False, stop=True)
                    vo = pA.tile([128, D], F32, tag="Avo")
                    nc.scalar.copy(vo[:], vp[:])
                    nc.scalar.dma_start(valN[rs, :], vo[:])

            # A2: DRAM->DRAM restructuring into the column-major x-pair
            # table. c0 pass: v(y,x) -> col x+1 els 0:32; c1 pass:
            # v(y,x) -> col x els 32:64.
            vnb = valN[:]
            if "noa" not in ablate:
                for h in range(NH):
                    for l, (H, W) in enumerate(SPATIAL):
                        src = bass.AP(
                            vnb.tensor, vnb.offset + LEVEL_START[l] * D + h * DH,
                            [[D, W], [W * D, H], [1, DH]])
                        dst0 = bass.AP(
                            vb.tensor,
                            vb.offset + (h * RPH + LRB[l] + RSPC[l] + 1) * 64,
                            [[RSPC[l] * 64, W], [64, H], [1, DH]])
                        nc.sync.dma_start(dst0, src)
                        dst1 = bass.AP(
                            vb.tensor,
                            vb.offset + (h * RPH + LRB[l] + 1) * 64 + DH,
                            [[RSPC[l] * 64, W], [64, H], [1, DH]])
                        nc.scalar.dma_start(dst1, src)

            # ---------------- Phase B: per-query-tile -------------------
            with (
                tc.tile_pool(name="pIn", bufs=3) as pIn,
                tc.tile_pool(name="pPr", bufs=2) as pPr,
                tc.tile_pool(name="pW4", bufs=3) as pW4,
                tc.tile_pool(name="pTw", bufs=4) as pTw,
                tc.tile_pool(name="pG", bufs=4) as pG,
                tc.tile_pool(name="pSW", bufs=3) as pSW,
                tc.tile_pool(name="pFin", bufs=2) as pFin,
                tc.tile_pool(name="psT", bufs=1, space="PSUM") as psT,
                tc.tile_pool(name="psMM", bufs=2, space="PSUM") as psMM,
                tc.tile_pool(name="psTw", bufs=2, space="PSUM") as psTw,
                tc.tile_pool(name="psO", bufs=2, space="PSUM") as psO,
                tc.tile_pool(name="psF", bufs=1, space="PSUM") as psF,
            ):

                def prep(i):
                    rs = slice(i * 128, (i + 1) * 128)
                    s = pIn.tile([128, D], F32, tag="Bs")
                    nc.sync.dma_start(s[:], srcq[rs, :])
                    p = pIn.tile([128, D], F32, tag="Bp")
                    nc.sync.dma_start(p[:], posq[rs, :])
                    r8 = pIn.tile([128, NL * 2], F32, tag="Br8")
                    nc.sync.dma_start(r8[:], refq[rs, :])

                    q = pPr.tile([128, D], F32, tag="Bq")
                    nc.vector.tensor_tensor(out=q[:], in0=s[:], in1=p[:], op=A.add)
                    qT = pPr.tile([128, 2, 128], F32, tag="BqT")
                    for k in range(2):
                        tp = psT.tile([128, 128], F32, tag="Btp")
                        nc.tensor.transpose(tp[:], q[:, k * 128:(k + 1) * 128], tid[:])
                        nc.scalar.copy(qT[:, k, :], tp[:])

                    qmm = psMM.tile([128, D + NT], F32, tag="Bqmm")
                    offp = qmm[:, 0:D]
                    nc.tensor.matmul(offp, lhsT=qT[:, 0, :], rhs=tWoff[:, 0:D], start=True, stop=False)
                    nc.tensor.matmul(offp, lhsT=qT[:, 1, :], rhs=tWoff[:, D:2 * D], start=False, stop=False)
                    nc.tensor.matmul(offp, lhsT=tones[:], rhs=tboff[:], start=False, stop=True)

                    attp = qmm[:, D:D + NT]
                    nc.tensor.matmul(attp, lhsT=qT[:, 0, :], rhs=tWattn[:, 0:NT], start=True, stop=False)
                    nc.tensor.matmul(attp, lhsT=qT[:, 1, :], rhs=tWattn[:, NT:2 * NT], start=False, stop=False)
                    nc.tensor.matmul(attp, lhsT=tones[:], rhs=tbattn[:], start=False, stop=True)

                    # softmax over the 16 (l,p) per head
                    mx = pPr.tile([128, NH], F32, tag="Bmx")
                    nc.vector.tensor_reduce(
                        out=mx[:], in_=_ap(qmm, D, [[16, NH], [1, 16]]),
                        axis=mybir.AxisListType.X, op=A.max)
                    xs = pPr.tile([128, NT], F32, tag="Bxs")
                    nc.vector.tensor_tensor(
                        out=xs[:], in0=attp,
                        in1=_ap(mx, 0, [[1, NH], [0, 16]]), op=A.subtract)
                    es = pPr.tile([128, NT], F32, tag="Bes")
                    nc.scalar.activation(es[:], xs[:], ACTF.Exp)
                    sm = pPr.tile([128, NH], F32, tag="Bsm")
                    nc.vector.tensor_reduce(
                        out=sm[:], in_=_ap(es, 0, [[16, NH], [1, 16]]),
                        axis=mybir.AxisListType.X, op=A.add)
                    rcp = pPr.tile([128, NH], F32, tag="Brcp")
                    nc.vector.reciprocal(rcp[:], sm[:])
                    aw = pPr.tile([128, NT], F32, tag="Baw")
                    nc.vector.tensor_tensor(
                        out=aw[:], in0=es[:],
                        in1=_ap(rcp, 0, [[1, NH], [0, 16]]), op=A.mult)

                    # sampling positions: px = (off - 0.5) + (ref*WH) broadcast
                    rsc = pPr.tile([128, NL * 2], F32, tag="Brsc")
                    nc.vector.tensor_tensor(out=rsc[:], in0=r8[:], in1=tdims8[:], op=A.mult)
                    r32 = pPr.tile([128, 32], F32, tag="Br32")
                    nc.vector.tensor_copy(out=r32[:], in_=_ap(rsc, 0, [[2, NL], [0, NP], [1, 2]]))
                    px = pPr.tile([128, D], F32, tag="Bpx")
                    nc.vector.scalar_tensor_tensor(
                        out=px[:], in0=offp, scalar=-0.5,
                        in1=_ap(r32, 0, [[0, NH], [1, 32]]), op0=A.add, op1=A.add)

                    # clip to [-1, dim]
                    xt = pPr.tile([128, NT], F32, tag="Bxt")
                    nc.vector.scalar_tensor_tensor(
                        out=xt[:], in0=_ap(px, 0, [[2, NT]]), scalar=-1.0,
                        in1=tcW[:], op0=A.max, op1=A.min)
                    yt = pPr.tile([128, NT], F32, tag="Byt")
                    nc.vector.scalar_tensor_tensor(
                        out=yt[:], in0=_ap(px, 1, [[2, NT]]), scalar=-1.0,
                        in1=tcH[:], op0=A.max, op1=A.min)

                    # floor + frac (round-to-int via 2^23 trick, then fix up)
                    def floor_frac(src, tagp):
                        r2 = pPr.tile([128, NT], F32, tag=tagp + "r2")
                        nc.scalar.activation(r2[:], src[:], ACTF.Identity, bias=t23[:, 0:1])
                        rn = pPr.tile([128, NT], F32, tag=tagp + "rn")
                        nc.scalar.activation(rn[:], r2[:], ACTF.Identity, bias=tm23[:, 0:1])
                        fx = pPr.tile([128, NT], F32, tag=tagp + "fx")
                        nc.vector.tensor_tensor(out=fx[:], in0=rn[:], in1=src[:], op=A.is_gt)
                        fl = pPr.tile([128, NT], F32, tag=tagp + "fl")
                        nc.vector.tensor_tensor(out=fl[:], in0=rn[:], in1=fx[:], op=A.subtract)
                        fr = pPr.tile([128, NT], F32, tag=tagp + "fr")
                        nc.vector.tensor_tensor(out=fr[:], in0=src[:], in1=fl[:], op=A.subtract)
                        return fl, fr

                    x0, dx = floor_frac(xt, "Bx")
                    y0, dy = floor_frac(yt, "By")

                    # corner weights with zero-padding masks
                    def corner_w(f0, dfrac, cM1, cM2, tagp):
                        inb1 = pPr.tile([128, NT], F32, tag=tagp + "i1")
                        nc.vector.tensor_tensor(out=inb1[:], in0=f0[:], in1=cM1[:], op=A.is_le)
                        m0 = pPr.tile([128, NT], F32, tag=tagp + "m0")
                        nc.vector.scalar_tensor_tensor(
                            out=m0[:], in0=f0[:], scalar=0.0, in1=inb1[:],
                            op0=A.is_ge, op1=A.mult)
                        m1 = pPr.tile([128, NT], F32, tag=tagp + "m1")
                        nc.vector.tensor_tensor(out=m1[:], in0=f0[:], in1=cM2[:], op=A.is_le)
                        om = pPr.tile([128, NT], F32, tag=tagp + "om")
                        nc.scalar.activation(om[:], dfrac[:], ACTF.Identity, bias=tone1[:, 0:1], scale=-1.0)
                        w0 = pPr.tile([128, NT], F32, tag=tagp + "w0")
                        nc.vector.tensor_tensor(out=w0[:], in0=om[:], in1=m0[:], op=A.mult)
                        w1 = pPr.tile([128, NT], F32, tag=tagp + "w1")
                        nc.vector.tensor_tensor(out=w1[:], in0=dfrac[:], in1=m1[:], op=A.mult)
                        return w0, w1

                    wx0, wx1 = corner_w(x0, dx, tcWm1, tcWm2, "BX")
                    wy0, wy1 = corner_w(y0, dy, tcHm1, tcHm2, "BY")

                    wy0a = pPr.tile([128, NT], F32, tag="Bwy0a")
                    nc.vector.tensor_tensor(out=wy0a[:], in0=wy0[:], in1=aw[:], op=A.mult)
                    wy1a = pPr.tile([128, NT], F32, tag="Bwy1a")
                    nc.vector.tensor_tensor(out=wy1a[:], in0=wy1[:], in1=aw[:], op=A.mult)

                    w4 = pW4.tile([128, 4 * NT], F32, tag="Bw4")
                    for jj, (wyj, wxk) in enumerate(
                        [(wy0a, wx0), (wy0a, wx1), (wy1a, wx0), (wy1a, wx1)]
                    ):
                        nc.vector.tensor_tensor(
                            out=_ap(w4, jj, [[4, NT]]), in0=wyj[:], in1=wxk[:], op=A.mult)

                    # gather row index: (x0c+1)*RSPC + (y0a+1) + head/level base
                    x0c = pPr.tile([128, NT], F32, tag="Bx0c")
                    nc.vector.tensor_tensor(out=x0c[:], in0=x0[:], in1=tcWm1[:], op=A.min)
                    y0a = pPr.tile([128, NT], F32, tag="By0a")
                    nc.vector.tensor_tensor(out=y0a[:], in0=y0[:], in1=tcHm1[:], op=A.min)
                    of1 = pPr.tile([128, NT], F32, tag="Bof1")
                    nc.vector.tensor_tensor(out=of1[:], in0=x0c[:], in1=tcRSPC[:], op=A.mult)
                    of2 = pPr.tile([128, NT], F32, tag="Bof2")
                    nc.vector.tensor_tensor(out=of2[:], in0=of1[:], in1=y0a[:], op=A.add)
                    offs = pPr.tile([128, NT], F32, tag="Boffs")
                    nc.vector.tensor_tensor(out=offs[:], in0=of2[:], in1=tcB2[:], op=A.add)

                    # wrapped idx tile: Tw[p, t*256 + j*8 + qh] = offs(16qh+p%16, t*32+j)
                    Tw = pTw.tile([128, 4 * 256], I16, tag="BTw")
                    Twb = Tw[:]
                    for qh in range(8):
                        po = psTw.tile([128, 128], F32, tag="Bpo")
                        nc.tensor.matmul(po[:], lhsT=tSall[:, qh * 128:(qh + 1) * 128],
                                         rhs=offs[:], start=True, stop=True)
                        nc.scalar.copy(
                            bass.AP(Twb.tensor, Twb.offset + qh,
                                    [list(Twb.ap[0]), [256, 4], [8, 32]]),
                            po[:])
                    return (rs, s, w4, Tw, offs if dbg else None, aw if dbg else None,
                            x0 if dbg else None, y0 if dbg else None)

                def sample(st):
                    rs, s, w4, Tw, d_offs_t, d_aw_t, d_x0_t, d_y0_t = st
                    vtb = value_t[:]
                    samp = pFin.tile([128, D], F32, tag="Bsamp")
                    for t in range(4):
                        g = pG.tile([128, 32, 128], F32, tag="Bg")
                        if "nogather" in ablate:
                            nc.vector.memset(g[:, 0, :], 0.0)
                        else:
                            nc.gpsimd.dma_gather(
                                out_ap=g[:],
                                in_ap=bass.AP(vtb.tensor, vtb.offset + t * 2 * RPH * 64,
                                              [[64, 2 * RPH], [1, 128]]),
                                idxs_ap=Tw[:, t * 256:(t + 1) * 256], num_idxs=4096,
                                num_idxs_reg=4096, elem_size=128, elem_step=64,
                                single_packet=False)
                        if "nosamp" in ablate:
                            nc.vector.memset(samp[:, t * 64:(t + 1) * 64], 0.0)
                            continue
                        QB = 4096
                        sw = pSW.tile([128, QB], BF16, tag="Bsw")
                        nc.vector.tensor_tensor(
                            out=_ap(sw, 0, [[32, 128], [1, 32]]),
                            in0=_ap(g, 0, [[32, 128], [1, 32]]),
                            in1=_ap(w4, t * 128, [[1, 128], [0, 32]]),
                            op=A.mult)
                        # in-place pairwise tree over the 128 32-el blocks
                        for n in (64, 32, 16, 8, 4):
                            nc.vector.tensor_tensor(
                                out=_ap(sw, 0, [[32, n], [1, 32]]),
                                in0=_ap(sw, 0, [[64, n], [1, 32]]),
                                in1=_ap(sw, 32, [[64, n], [1, 32]]), op=A.add)
                        nc.vector.tensor_tensor(
                            out=samp[:, t * 64:(t + 1) * 64],
                            in0=_ap(sw, 0, [[64, 2], [1, 32]]),
                            in1=_ap(sw, 32, [[64, 2], [1, 32]]), op=A.add)

                    # output projection
                    sT = pFin.tile([128, 2, 128], F32, tag="BsT")
                    for k in range(2):
                        tp = psT.tile([128, 128], F32, tag="Btp")
                        nc.tensor.transpose(tp[:], samp[:, k * 128:(k + 1) * 128], tid[:])
                        nc.scalar.copy(sT[:, k, :], tp[:])
                    o2p = psO.tile([128, D], F32, tag="Bo23")
                    nc.tensor.matmul(o2p[:], lhsT=sT[:, 0, :], rhs=tWout[:, 0:D], start=True, stop=False)
                    nc.tensor.matmul(o2p[:], lhsT=sT[:, 1, :], rhs=tWout[:, D:2 * D], start=False, stop=False)
                    nc.tensor.matmul(o2p[:], lhsT=tones[:], rhs=tbout[:], start=False, stop=True)

                    # residual + layernorm
                    def layer_norm(inp_sbuf, res_psum, gt, bt, tagp):
                        x1 = pFin.tile([128, D], F32, tag="BLx1")
                        sums = pFin.tile([128, 1], F32, tag="BLsu")
                        nc.vector.scalar_tensor_tensor(
                            out=x1[:], in0=inp_sbuf[:], scalar=0.0, in1=res_psum[:],
                            op0=A.add, op1=A.add, accum_out=sums[:])
                        negm = pFin.tile([128, 1], F32, tag="BLnm")
                        nc.scalar.mul(negm[:], sums[:], -1.0 / D)
                        sq = pFin.tile([128, D], F32, tag="BLsq")
                        ssq = pFin.tile([128, 1], F32, tag="BLss")
                        nc.scalar.activation(sq[:], x1[:], ACTF.Square,
                                             bias=negm[:, 0:1], accum_out=ssq[:])
                        sd = pFin.tile([128, 1], F32, tag="BLsd")
                        nc.scalar.activation(sd[:], ssq[:], ACTF.Sqrt,
                                             scale=1.0 / D, bias=teps[:, 0:1])
                        rstd = pFin.tile([128, 1], F32, tag="BLrs")
                        nc.vector.reciprocal(rstd[:], sd[:])
                        xh = pFin.tile([128, D], F32, tag="BLxh")
                        nc.vector.tensor_scalar(
                            out=xh[:], in0=x1[:], scalar1=negm[:, 0:1],
                            scalar2=rstd[:, 0:1], op0=A.add, op1=A.mult)
                        yv = pFin.tile([128, D], F32, tag=tagp + "y")
                        nc.vector.tensor_tensor(out=yv[:], in0=xh[:], in1=gt[:], op=A.mult)
                        nc.vector.tensor_tensor(out=yv[:], in0=yv[:], in1=bt[:], op=A.add)
                        return yv

                    y1v = layer_norm(s, o2p, tg1, tbe1, "BL1")

                    # FFN
                    yT = pFin.tile([128, 2, 128], BF16, tag="ByT")
                    for k in range(2):
                        tp = psT.tile([128, 128], F32, tag="Btp")
                        nc.tensor.transpose(tp[:], y1v[:, k * 128:(k + 1) * 128], tid[:])
                        nc.scalar.copy(yT[:, k, :], tp[:])
                    h1 = pFin.tile([128, DFF], BF16, tag="Bh1")
                    for j in range(8):
                        js = slice(j * 128, (j + 1) * 128)
                        hp = psF.tile([128, 128], F32, tag="Bhp")
                        nc.tensor.matmul(hp[:], lhsT=tW1[:, 0 * DFF + j * 128:0 * DFF + (j + 1) * 128],
                                         rhs=yT[:, 0, :], start=True, stop=False)
                        nc.tensor.matmul(hp[:], lhsT=tW1[:, 1 * DFF + j * 128:1 * DFF + (j + 1) * 128],
                                         rhs=yT[:, 1, :], start=False, stop=False)
                        nc.tensor.matmul(hp[:], lhsT=tb1b[:, js], rhs=tonesb[:], start=False, stop=True)
                        nc.scalar.activation(h1[:, js], hp[:], ACTF.Relu)
                    o3p = psO.tile([128, D], F32, tag="Bo23")
                    for j in range(8):
                        js = slice(j * 128, (j + 1) * 128)
                        nc.tensor.matmul(o3p[:], lhsT=h1[:, js], rhs=tW2[:, j * D:(j + 1) * D],
                                         start=(j == 0), stop=False)
                    nc.tensor.matmul(o3p[:], lhsT=tonesb[:], rhs=tb2b[:], start=False, stop=True)

                    y2v = layer_norm(y1v, o3p, tg2, tbe2, "BL2")
                    nc.sync.dma_start(outq[rs, :], y2v[:])
                    if dbg:
                        nc.sync.dma_start(d_ofs[rs, :], d_offs_t[:])
                        nc.sync.dma_start(d_aw[rs, :], d_aw_t[:])
                        nc.sync.dma_start(d_w4[rs, :], w4[:])
                        nc.sync.dma_start(d_samp[rs, :], samp[:])
                        nc.sync.dma_start(d_x0[rs, :], d_x0_t[:])
                        nc.sync.dma_start(d_y0[rs, :], d_y0_t[:])

                # 2-stage software pipeline: prep(i+1) issues before
                # sample(i) so every engine's FIFO keeps the gather
                # stream fed.
                ntiles = 0 if "nob" in ablate else N_Q_TILES
                state = None
                for i in range(ntiles + 1):
                    new = prep(i) if i < ntiles else None
                    if state is not None:
                        sample(state)
                    state = new

    nc.compile()
    return nc


# ----------------------------------------------------------------------
# host-side wrapper
# ----------------------------------------------------------------------
_NC_CACHE = None


def _get_nc():
    global _NC_CACHE
    if _NC_CACHE is None:
        _NC_CACHE = build()
    return _NC_CACHE


def make_consts():
    h_i, l_i, p_i = np.meshgrid(np.arange(NH), np.arange(NL), np.arange(NP), indexing="ij")
    Wl = np.array([w for (_, w) in SPATIAL], np.float32)
    Hl = np.array([h for (h, _) in SPATIAL], np.float32)
    lw = Wl[l_i].reshape(-1)
    lh = Hl[l_i].reshape(-1)
    rspc = np.array(RSPC, np.float32)[l_i].reshape(-1)
    lrb = np.array(LRB, np.float32)[l_i].reshape(-1)
    b2 = ((h_i % 2) * RPH).reshape(-1) + lrb + rspc + 1.0
    rep = lambda v: np.tile(v[None, :].astype(np.float32), (128, 1))
    dims8 = np.zeros(NL * 2, np.float32)
    dims8[0::2] = Wl
    dims8[1::2] = Hl
    Sall = np.zeros((128, 8 * 128), np.float32)
    for qh in range(8):
        for q16 in range(16):
            for k in range(8):
                Sall[16 * qh + q16, qh * 128 + 16 * k + q16] = 1.0
    return {
        "cW": rep(lw), "cH": rep(lh),
        "cWm1": rep(lw - 1), "cHm1": rep(lh - 1),
        "cWm2": rep(lw - 2), "cHm2": rep(lh - 2),
        "cRSPC": rep(rspc), "cB2": rep(b2),
        "dims8": rep(dims8),
        "ident": np.eye(128, dtype=np.float32),
        "ones_row": np.ones((1, 128), np.float32),
        "Sall": Sall,
        "zeros8k": np.zeros((1, 8192), np.float32),
    }


SHARD_STARTS = [0, 3324, 6648, 9972]
SHARD_SIZES = [3324, 3324, 3324, 3322]


def make_in_maps(inputs):
    consts = make_consts()
    in_maps = []
    for core in range(8):
        b, c = core // 4, core % 4
        st, sz = SHARD_STARTS[c], SHARD_SIZES[c]
        src_full = np.zeros((PAD_LEN, D), np.float32)
        src_full[:LEN] = inputs["src"][b]
        srcq = np.zeros((Q_SH, D), np.float32)
        srcq[:sz] = inputs["src"][b, st:st + sz]
        posq = np.zeros((Q_SH, D), np.float32)
        posq[:sz] = inputs["pos"][b, st:st + sz]
        refq = np.full((Q_SH, NL * 2), 0.5, np.float32)
        refq[:sz] = inputs["reference_points"][b, st:st + sz].reshape(sz, NL * 2)
        m = {
            "src_full": src_full, "srcq": srcq, "posq": posq, "refq": refq,
            "Wv": inputs["W_value"], "Woff": inputs["W_off"],
            "Wattn": inputs["W_attn"], "Wout": inputs["W_out"],
            "W1": inputs["W1"], "W2": inputs["W2"],
            "bv": inputs["b_value"][None, :], "boff": inputs["b_off"][None, :],
            "battn": inputs["b_attn"][None, :], "bout": inputs["b_out"][None, :],
            "b1": inputs["b1"][None, :], "b2": inputs["b2"][None, :],
            "g1r": np.tile(inputs["g1"][None, :], (128, 1)),
            "be1r": np.tile(inputs["be1"][None, :], (128, 1)),
            "g2r": np.tile(inputs["g2"][None, :], (128, 1)),
            "be2r": np.tile(inputs["be2"][None, :], (128, 1)),
        }
        for k in ("cW", "cH", "cWm1", "cHm1", "cWm2", "cHm2", "cRSPC", "cB2",
                  "dims8", "ident", "ones_row", "Sall", "zeros8k"):
            m[k] = consts[k]
        import ml_dtypes
        bf16_params = {"Wv", "W1", "W2"}
        in_maps.append({
            k: np.ascontiguousarray(v, ml_dtypes.bfloat16 if k in bf16_params else np.float32)
            for k, v in m.items()})
    return in_maps


def assemble_out(results):
    out = np.empty((2, LEN, D), np.float32)
    for core in range(8):
        b, c = core // 4, core % 4
        st, sz = SHARD_STARTS[c], SHARD_SIZES[c]
        out[b, st:st + sz] = results[core]["outq"][:sz]
    return out


def run(inputs, trace=False, **kw):
    nc = _get_nc()
    in_maps = make_in_maps(inputs)
    res = run_bass_kernel_spmd(nc, in_maps, core_ids=list(range(8)), trace=trace, **kw)
    return assemble_out(res.results), res


def kernel(**inputs):
    out, _ = run(inputs)
    return out


# revision 14
# speedup vs baseline: 1.0860x; 1.0564x over previous
"""Deformable-DETR transformer encoder layer on 8 Trainium2 NeuronCores.

Sharding: data-parallel over batch (B=2 -> 4 cores per batch element),
sequence-parallel over queries within the batch group.

Value memory layout: per (head, level) the x-pair rows [v(y,x)|v(y,x+1)]
are stored COLUMN-major (row index = (x+1)*(H+2) + (y+1)), so rows j and
j+1 hold all 4 bilinear corners of one sample point. One dma_gather index
per point (elem_size=128 f32, elem_step=64 overlapping rows) halves the
SWDGE descriptor-generation cost vs. a per-corner-pair gather.

Self-contained: hardcodes all shapes/constants from the problem spec.
"""

import numpy as np

import concourse.bass as bass
import concourse.mybir as mybir
import concourse.tile as tile
from concourse import bacc
from concourse.bass_utils import run_bass_kernel_spmd

F32 = mybir.dt.float32
I32 = mybir.dt.int32
I16 = mybir.dt.int16
BF16 = mybir.dt.bfloat16

# ---- problem constants -------------------------------------------------
SPATIAL = [(100, 100), (50, 50), (25, 25), (13, 13)]
LEVEL_START = [0, 10000, 12500, 13125]
LEN = 13294
D = 256
NH = 8
NL = 4
NP = 4
DH = 32
DFF = 1024
EPS = 1e-5

PAD_LEN = 13312           # 104 * 128, full-sequence padded length
N_FULL_TILES = PAD_LEN // 128
Q_SH = 3328               # 26 * 128, per-core query shard (padded)
N_Q_TILES = Q_SH // 128

# column-major x-pair value table geometry (per head)
RSPC = [h + 2 for h, w in SPATIAL]        # rows per column = H+2
TCOLS = [w + 1 for h, w in SPATIAL]       # columns = W+1 (x in [-1, W-1])
LRB = [0]
for l in range(NL):
    LRB.append(LRB[-1] + TCOLS[l] * RSPC[l])
RPH = LRB[-1]                              # rows per head = 13866
LRB = LRB[:-1]
TBL_ROWS = NH * RPH + 2                    # +2 pad rows for tail reads
TBL_ELEMS = TBL_ROWS * 64

NT = NH * NL * NP         # 128 (h,l,p) triples
GIDX = NT * 128           # idx per tile = 128 q * 128 points (4 calls x 4096)

TWO23 = float(3 << 22)  # 1.5*2^23 magic round constant


def _ap(t, offset_elems, dims):
    """Custom free-dim AP view of an SBUF tile (keeps full 128 partitions)."""
    base = t[:]
    return bass.AP(base.tensor, base.offset + offset_elems, [list(base.ap[0])] + [list(d) for d in dims])


def build(dbg=False, ablate=()):
    nc = bacc.Bacc("TRN2", target_bir_lowering=False, debug=False, num_devices=8)
    A = mybir.AluOpType
    ACTF = mybir.ActivationFunctionType

    def param(name, shape, dtype=F32, out=False):
        return nc.declare_dram_parameter(name, list(shape), dtype, isOutput=out)

    src_full = param("src_full", [PAD_LEN, D])
    srcq = param("srcq", [Q_SH, D])
    posq = param("posq", [Q_SH, D])
    refq = param("refq", [Q_SH, NL * 2])
    Wv = param("Wv", [D, D], BF16)
    Woff = param("Woff", [D, D])
    Wattn = param("Wattn", [D, NT])
    Wout = param("Wout", [D, D])
    W1 = param("W1", [D, DFF], BF16)
    W2 = param("W2", [DFF, D], BF16)
    bv = param("bv", [1, D])
    boff = param("boff", [1, D])
    battn = param("battn", [1, NT])
    bout = param("bout", [1, D])
    b1 = param("b1", [1, DFF])
    b2 = param("b2", [1, D])
    g1r = param("g1r", [128, D])
    be1r = param("be1r", [128, D])
    g2r = param("g2r", [128, D])
    be2r = param("be2r", [128, D])
    ident = param("ident", [128, 128])
    ones_row = param("ones_row", [1, 128])
    cW = param("cW", [128, NT])
    cH = param("cH", [128, NT])
    cWm1 = param("cWm1", [128, NT])
    cHm1 = param("cHm1", [128, NT])
    cWm2 = param("cWm2", [128, NT])
    cHm2 = param("cHm2", [128, NT])
    cRSPC = param("cRSPC", [128, NT])
    cB2 = param("cB2", [128, NT])
    dims8 = param("dims8", [128, NL * 2])
    Sall = param("Sall", [128, 8 * 128])
    zeros8k = param("zeros8k", [1, 8192])
    outq = param("outq", [Q_SH, D], out=True)
    if dbg:
        d_ofs = param("d_ofs", [Q_SH, NT], out=True)
        d_aw = param("d_aw", [Q_SH, NT], out=True)
        d_w4 = param("d_w4", [Q_SH, 4 * NT], out=True)
        d_samp = param("d_samp", [Q_SH, D], out=True)
        d_x0 = param("d_x0", [Q_SH, NT], out=True)
        d_y0 = param("d_y0", [Q_SH, NT], out=True)

    with tile.TileContext(nc) as tc:
        with (
            tc.tile_pool(name="const", bufs=1) as cp,
            tc.tile_pool(name="dram", bufs=1, space="DRAM") as dp,
        ):
            valN = dp.tile([PAD_LEN, D], F32, tag="valN")
            value_t = dp.tile([TBL_ROWS, 64], F32, tag="value")

            def cload(src_ap, p, n, tag):
                t = cp.tile([p, n], F32, tag=tag)
                nc.sync.dma_start(t[:], src_ap[:])
                return t

            tWv = cp.tile([128, 2 * D], BF16, tag="tWv")
            nc.sync.dma_start(tWv[:, 0:D], Wv[0:128, :])
            nc.sync.dma_start(tWv[:, D:2 * D], Wv[128:256, :])
            tid = cload(ident, 128, 128, "tid")
            tbv = cload(bv, 1, D, "tbv")
            tones = cload(ones_row, 1, 128, "tones")
            tidb = cp.tile([128, 128], BF16, tag="tidb")
            nc.vector.tensor_copy(out=tidb[:], in_=tid[:])
            tonesb = cp.tile([1, 128], BF16, tag="tonesb")
            nc.vector.tensor_copy(out=tonesb[:], in_=tones[:])
            tbvb = cp.tile([1, D], BF16, tag="tbvb")
            nc.vector.tensor_copy(out=tbvb[:], in_=tbv[:])
            tWoff = cp.tile([128, 2 * D], F32, tag="tWoff")
            nc.sync.dma_start(tWoff[:, 0:D], Woff[0:128, :])
            nc.sync.dma_start(tWoff[:, D:2 * D], Woff[128:256, :])
            tWattn = cp.tile([128, 2 * NT], F32, tag="tWattn")
            nc.sync.dma_start(tWattn[:, 0:NT], Wattn[0:128, :])
            nc.sync.dma_start(tWattn[:, NT:2 * NT], Wattn[128:256, :])
            tWout = cp.tile([128, 2 * D], F32, tag="tWout")
            nc.sync.dma_start(tWout[:, 0:D], Wout[0:128, :])
            nc.sync.dma_start(tWout[:, D:2 * D], Wout[128:256, :])
            tW1 = cp.tile([128, 2 * DFF], BF16, tag="tW1")
            nc.sync.dma_start(tW1[:, 0:DFF], W1[0:128, :])
            nc.sync.dma_start(tW1[:, DFF:2 * DFF], W1[128:256, :])
            tW2 = cp.tile([128, 8 * D], BF16, tag="tW2")
            for j in range(8):
                nc.sync.dma_start(tW2[:, j * D:(j + 1) * D], W2[j * 128:(j + 1) * 128, :])

            tboff = cload(boff, 1, D, "tboff")
            tbattn = cload(battn, 1, NT, "tbattn")
            tbout = cload(bout, 1, D, "tbout")
            tb1 = cload(b1, 1, DFF, "tb1")
            tb2 = cload(b2, 1, D, "tb2")
            tg1 = cload(g1r, 128, D, "tg1")
            tbe1 = cload(be1r, 128, D, "tbe1")
            tg2 = cload(g2r, 128, D, "tg2")
            tbe2 = cload(be2r, 128, D, "tbe2")
            tcW = cload(cW, 128, NT, "tcW")
            tcH = cload(cH, 128, NT, "tcH")
            tcWm1 = cload(cWm1, 128, NT, "tcWm1")
            tcHm1 = cload(cHm1, 128, NT, "tcHm1")
            tcWm2 = cload(cWm2, 128, NT, "tcWm2")
            tcHm2 = cload(cHm2, 128, NT, "tcHm2")
            tcRSPC = cload(cRSPC, 128, NT, "tcRSPC")
            tcB2 = cload(cB2, 128, NT, "tcB2")
            tdims8 = cload(dims8, 128, NL * 2, "tdims8")
            tSall = cload(Sall, 128, 8 * 128, "tSall")

            # bf16 copies for mixed-precision matmuls
            tb1b = cp.tile([1, DFF], BF16, tag="tb1b")
            nc.vector.tensor_copy(out=tb1b[:], in_=tb1[:])
            tb2b = cp.tile([1, D], BF16, tag="tb2b")
            nc.vector.tensor_copy(out=tb2b[:], in_=tb2[:])

            # small scalar constants for ACT bias operands
            def cconst(val, tag):
                t = cp.tile([128, 1], F32, tag=tag)
                nc.vector.memset(t[:], val)
                return t

            t23 = cconst(TWO23, "t23")
            tm23 = cconst(-TWO23, "tm23")
            tone1 = cconst(1.0, "tone1")
            teps = cconst(EPS, "teps")

            # ---------------- Phase A: value table ---------------------
            # A0: zero the pad regions the c0/c1 passes never write (they
            # are weight-masked but must be finite): row 0 and row H+1 of
            # every column, col 0 els 0:32, col W els 32:64, tail pad rows.
            vb = value_t[:]
            zb = zeros8k[0:1, :]
            zeng = [nc.sync, nc.scalar]
            for l, (H, W) in enumerate(SPATIAL):
                e = zeng[l % 2]
                for r0 in (0, H + 1):
                    e.dma_start(
                        bass.AP(vb.tensor, vb.offset + (LRB[l] + r0) * 64,
                                [[RPH * 64, NH], [RSPC[l] * 64, W + 1], [1, 64]]),
                        bass.AP(zb.tensor, zb.offset,
                                [[0, NH], [0, W + 1], [1, 64]]))
                e.dma_start(
                    bass.AP(vb.tensor, vb.offset + LRB[l] * 64,
                            [[RPH * 64, NH], [64, RSPC[l]], [1, 32]]),
                    bass.AP(zb.tensor, zb.offset, [[0, NH], [0, RSPC[l]], [1, 32]]))
                e.dma_start(
                    bass.AP(vb.tensor, vb.offset + (LRB[l] + W * RSPC[l]) * 64 + 32,
                            [[RPH * 64, NH], [64, RSPC[l]], [1, 32]]),
                    bass.AP(zb.tensor, zb.offset, [[0, NH], [0, RSPC[l]], [1, 32]]))
            nc.sync.dma_start(
                bass.AP(vb.tensor, vb.offset + NH * RPH * 64, [[1, 128]]),
                bass.AP(zb.tensor, zb.offset, [[1, 128]]))

            # A2 (def): DRAM->DRAM restructuring into the column-major
            # x-pair table, emitted per level from inside the pass1 loop
            # as soon as its valN rows are complete. c0 pass: v(y,x) ->
            # col x+1 els 0:32; c1 pass: v(y,x) -> col x els 32:64.
            vnb = valN[:]

            def emit_pass2(levels):
                for h in range(NH):
                    for l in levels:
                        H, W = SPATIAL[l]
                        src = bass.AP(
                            vnb.tensor, vnb.offset + LEVEL_START[l] * D + h * DH,
                            [[D, W], [W * D, H], [1, DH]])
                        dst0 = bass.AP(
                            vb.tensor,
                            vb.offset + (h * RPH + LRB[l] + RSPC[l] + 1) * 64,
                            [[RSPC[l] * 64, W], [64, H], [1, DH]])
                        nc.sync.dma_start(dst0, src)
                        dst1 = bass.AP(
                            vb.tensor,
                            vb.offset + (h * RPH + LRB[l] + 1) * 64 + DH,
                            [[RSPC[l] * 64, W], [64, H], [1, DH]])
                        nc.scalar.dma_start(dst1, src)

            # A1: natural-layout value projection valN = src @ Wv + bv
            with (
                tc.tile_pool(name="pA", bufs=6) as pA,
                tc.tile_pool(name="psA", bufs=2, space="PSUM") as psA,
                tc.tile_pool(name="psA2", bufs=3, space="PSUM") as psA2,
            ):
                for i in range(0 if "noa" in ablate else N_FULL_TILES):
                    rs = slice(i * 128, (i + 1) * 128)
                    s = pA.tile([128, D], F32, tag="As")
                    nc.sync.dma_start(s[:], src_full[rs, :])
                    sb = pA.tile([128, D], BF16, tag="Asb")
                    nc.vector.tensor_copy(out=sb[:], in_=s[:])
                    sT = pA.tile([128, 2, 128], BF16, tag="AsT")
                    for k in range(2):
                        tp = psA.tile([128, 128], BF16, tag="Atp")
                        nc.tensor.transpose(tp[:], sb[:, k * 128:(k + 1) * 128], tidb[:])
                        nc.vector.tensor_copy(out=sT[:, k, :], in_=tp[:])
                    vp = psA2.tile([128, D], F32, tag="Avp")
                    nc.tensor.matmul(vp[:], lhsT=sT[:, 0, :], rhs=tWv[:, 0:D], start=True, stop=False)
                    nc.tensor.matmul(vp[:], lhsT=sT[:, 1, :], rhs=tWv[:, D:2 * D], start=False, stop=False)
                    nc.tensor.matmul(vp[:], lhsT=tonesb[:], rhs=tbvb[:], start=False, stop=True)
                    vo = pA.tile([128, D], F32, tag="Avo")
                    nc.scalar.copy(vo[:], vp[:])
                    nc.scalar.dma_start(valN[rs, :], vo[:])
                    if "noa" not in ablate:
                        if i == 79:
                            emit_pass2([0])
                        elif i == 98:
                            emit_pass2([1])
                        elif i == N_FULL_TILES - 1:
                            emit_pass2([2, 3])

            # ---------------- Phase B: per-query-tile -------------------
            with (
                tc.tile_pool(name="pS", bufs=5) as pS,
                tc.tile_pool(name="pP", bufs=2) as pP,
                tc.tile_pool(name="pPr", bufs=2) as pPr,
                tc.tile_pool(name="pW4", bufs=5) as pW4,
                tc.tile_pool(name="pTw", bufs=5) as pTw,
                tc.tile_pool(name="pG", bufs=4) as pG,
                tc.tile_pool(name="pSW", bufs=2) as pSW,
                tc.tile_pool(name="pFin", bufs=2) as pFin,
                tc.tile_pool(name="psT", bufs=1, space="PSUM") as psT,
                tc.tile_pool(name="psMM", bufs=2, space="PSUM") as psMM,
                tc.tile_pool(name="psTw", bufs=2, space="PSUM") as psTw,
                tc.tile_pool(name="psO", bufs=2, space="PSUM") as psO,
                tc.tile_pool(name="psF", bufs=1, space="PSUM") as psF,
            ):

                def prep(i):
                    rs = slice(i * 128, (i + 1) * 128)
                    s = pS.tile([128, D], F32, tag="Bs")
                    nc.sync.dma_start(s[:], srcq[rs, :])
                    p = pP.tile([128, D], F32, tag="Bp")
                    nc.sync.dma_start(p[:], posq[rs, :])
                    r8 = pP.tile([128, NL * 2], F32, tag="Br8")
                    nc.sync.dma_start(r8[:], refq[rs, :])

                    q = pPr.tile([128, D], F32, tag="Bq")
                    nc.vector.tensor_tensor(out=q[:], in0=s[:], in1=p[:], op=A.add)
                    qT = pPr.tile([128, 2, 128], F32, tag="BqT")
                    for k in range(2):
                        tp = psT.tile([128, 128], F32, tag="Btp")
                        nc.tensor.transpose(tp[:], q[:, k * 128:(k + 1) * 128], tid[:])
                        nc.scalar.copy(qT[:, k, :], tp[:])

                    qmm = psMM.tile([128, D + NT], F32, tag="Bqmm")
                    offp = qmm[:, 0:D]
                    nc.tensor.matmul(offp, lhsT=qT[:, 0, :], rhs=tWoff[:, 0:D], start=True, stop=False)
                    nc.tensor.matmul(offp, lhsT=qT[:, 1, :], rhs=tWoff[:, D:2 * D], start=False, stop=False)
                    nc.tensor.matmul(offp, lhsT=tones[:], rhs=tboff[:], start=False, stop=True)

                    attp = qmm[:, D:D + NT]
                    nc.tensor.matmul(attp, lhsT=qT[:, 0, :], rhs=tWattn[:, 0:NT], start=True, stop=False)
                    nc.tensor.matmul(attp, lhsT=qT[:, 1, :], rhs=tWattn[:, NT:2 * NT], start=False, stop=False)
                    nc.tensor.matmul(attp, lhsT=tones[:], rhs=tbattn[:], start=False, stop=True)

                    # softmax over the 16 (l,p) per head
                    mx = pPr.tile([128, NH], F32, tag="Bmx")
                    nc.vector.tensor_reduce(
                        out=mx[:], in_=_ap(qmm, D, [[16, NH], [1, 16]]),
                        axis=mybir.AxisListType.X, op=A.max)
                    xs = pPr.tile([128, NT], F32, tag="Bxs")
                    nc.vector.tensor_tensor(
                        out=xs[:], in0=attp,
                        in1=_ap(mx, 0, [[1, NH], [0, 16]]), op=A.subtract)
                    es = pPr.tile([128, NT], F32, tag="Bes")
                    nc.scalar.activation(es[:], xs[:], ACTF.Exp)
                    sm = pPr.tile([128, NH], F32, tag="Bsm")
                    nc.vector.tensor_reduce(
                        out=sm[:], in_=_ap(es, 0, [[16, NH], [1, 16]]),
                        axis=mybir.AxisListType.X, op=A.add)
                    rcp = pPr.tile([128, NH], F32, tag="Brcp")
                    nc.vector.reciprocal(rcp[:], sm[:])
                    aw = pPr.tile([128, NT], F32, tag="Baw")
                    nc.vector.tensor_tensor(
                        out=aw[:], in0=es[:],
                        in1=_ap(rcp, 0, [[1, NH], [0, 16]]), op=A.mult)

                    # sampling positions: px = (off - 0.5) + (ref*WH) broadcast
                    rsc = pPr.tile([128, NL * 2], F32, tag="Brsc")
                    nc.vector.tensor_tensor(out=rsc[:], in0=r8[:], in1=tdims8[:], op=A.mult)
                    r32 = pPr.tile([128, 32], F32, tag="Br32")
                    nc.vector.tensor_copy(out=r32[:], in_=_ap(rsc, 0, [[2, NL], [0, NP], [1, 2]]))
                    px = pPr.tile([128, D], F32, tag="Bpx")
                    nc.vector.scalar_tensor_tensor(
                        out=px[:], in0=offp, scalar=-0.5,
                        in1=_ap(r32, 0, [[0, NH], [1, 32]]), op0=A.add, op1=A.add)

                    # clip to [-1, dim]
                    xt = pPr.tile([128, NT], F32, tag="Bxt")
                    nc.vector.scalar_tensor_tensor(
                        out=xt[:], in0=_ap(px, 0, [[2, NT]]), scalar=-1.0,
                        in1=tcW[:], op0=A.max, op1=A.min)
                    yt = pPr.tile([128, NT], F32, tag="Byt")
                    nc.vector.scalar_tensor_tensor(
                        out=yt[:], in0=_ap(px, 1, [[2, NT]]), scalar=-1.0,
                        in1=tcH[:], op0=A.max, op1=A.min)

                    # floor + frac (round-to-int via 2^23 trick, then fix up)
                    def floor_frac(src, tagp):
                        r2 = pPr.tile([128, NT], F32, tag=tagp + "r2")
                        nc.scalar.activation(r2[:], src[:], ACTF.Identity, bias=t23[:, 0:1])
                        rn = pPr.tile([128, NT], F32, tag=tagp + "rn")
                        nc.scalar.activation(rn[:], r2[:], ACTF.Identity, bias=tm23[:, 0:1])
                        fx = pPr.tile([128, NT], F32, tag=tagp + "fx")
                        nc.vector.tensor_tensor(out=fx[:], in0=rn[:], in1=src[:], op=A.is_gt)
                        fl = pPr.tile([128, NT], F32, tag=tagp + "fl")
                        nc.vector.tensor_tensor(out=fl[:], in0=rn[:], in1=fx[:], op=A.subtract)
                        fr = pPr.tile([128, NT], F32, tag=tagp + "fr")
                        nc.vector.tensor_tensor(out=fr[:], in0=src[:], in1=fl[:], op=A.subtract)
                        return fl, fr

                    x0, dx = floor_frac(xt, "Bx")
                    y0, dy = floor_frac(yt, "By")

                    # corner weights with zero-padding masks
                    def corner_w(f0, dfrac, cM1, cM2, tagp):
                        inb1 = pPr.tile([128, NT], F32, tag=tagp + "i1")
                        nc.vector.tensor_tensor(out=inb1[:], in0=f0[:], in1=cM1[:], op=A.is_le)
                        m0 = pPr.tile([128, NT], F32, tag=tagp + "m0")
                        nc.vector.scalar_tensor_tensor(
                            out=m0[:], in0=f0[:], scalar=0.0, in1=inb1[:],
                            op0=A.is_ge, op1=A.mult)
                        m1 = pPr.tile([128, NT], F32, tag=tagp + "m1")
                        nc.vector.tensor_tensor(out=m1[:], in0=f0[:], in1=cM2[:], op=A.is_le)
                        om = pPr.tile([128, NT], F32, tag=tagp + "om")
                        nc.scalar.activation(om[:], dfrac[:], ACTF.Identity, bias=tone1[:, 0:1], scale=-1.0)
                        w0 = pPr.tile([128, NT], F32, tag=tagp + "w0")
                        nc.vector.tensor_tensor(out=w0[:], in0=om[:], in1=m0[:], op=A.mult)
                        w1 = pPr.tile([128, NT], F32, tag=tagp + "w1")
                        nc.vector.tensor_tensor(out=w1[:], in0=dfrac[:], in1=m1[:], op=A.mult)
                        return w0, w1

                    wx0, wx1 = corner_w(x0, dx, tcWm1, tcWm2, "BX")
                    wy0, wy1 = corner_w(y0, dy, tcHm1, tcHm2, "BY")

                    wy0a = pPr.tile([128, NT], F32, tag="Bwy0a")
                    nc.vector.tensor_tensor(out=wy0a[:], in0=wy0[:], in1=aw[:], op=A.mult)
                    wy1a = pPr.tile([128, NT], F32, tag="Bwy1a")
                    nc.vector.tensor_tensor(out=wy1a[:], in0=wy1[:], in1=aw[:], op=A.mult)

                    w4 = pW4.tile([128, 4 * NT], F32, tag="Bw4")
                    for jj, (wyj, wxk) in enumerate(
                        [(wy0a, wx0), (wy0a, wx1), (wy1a, wx0), (wy1a, wx1)]
                    ):
                        nc.vector.tensor_tensor(
                            out=_ap(w4, jj, [[4, NT]]), in0=wyj[:], in1=wxk[:], op=A.mult)

                    # gather row index: (x0c+1)*RSPC + (y0a+1) + head/level base
                    x0c = pPr.tile([128, NT], F32, tag="Bx0c")
                    nc.vector.tensor_tensor(out=x0c[:], in0=x0[:], in1=tcWm1[:], op=A.min)
                    y0a = pPr.tile([128, NT], F32, tag="By0a")
                    nc.vector.tensor_tensor(out=y0a[:], in0=y0[:], in1=tcHm1[:], op=A.min)
                    of1 = pPr.tile([128, NT], F32, tag="Bof1")
                    nc.vector.tensor_tensor(out=of1[:], in0=x0c[:], in1=tcRSPC[:], op=A.mult)
                    of2 = pPr.tile([128, NT], F32, tag="Bof2")
                    nc.vector.tensor_tensor(out=of2[:], in0=of1[:], in1=y0a[:], op=A.add)
                    offs = pPr.tile([128, NT], F32, tag="Boffs")
                    nc.vector.tensor_tensor(out=offs[:], in0=of2[:], in1=tcB2[:], op=A.add)

                    # wrapped idx tile: Tw[p, t*256 + j*8 + qh] = offs(16qh+p%16, t*32+j)
                    Tw = pTw.tile([128, 4 * 256], I16, tag="BTw")
                    Twb = Tw[:]
                    for qh in range(8):
                        po = psTw.tile([128, 128], F32, tag="Bpo")
                        nc.tensor.matmul(po[:], lhsT=tSall[:, qh * 128:(qh + 1) * 128],
                                         rhs=offs[:], start=True, stop=True)
                        nc.scalar.copy(
                            bass.AP(Twb.tensor, Twb.offset + qh,
                                    [list(Twb.ap[0]), [256, 4], [8, 32]]),
                            po[:])
                    return (rs, s, w4, Tw, offs if dbg else None, aw if dbg else None,
                            x0 if dbg else None, y0 if dbg else None)

                def sample(st):
                    rs, s, w4, Tw, d_offs_t, d_aw_t, d_x0_t, d_y0_t = st
                    vtb = value_t[:]
                    samp = pFin.tile([128, D], F32, tag="Bsamp")
                    for t in range(4):
                        g = pG.tile([128, 32, 128], F32, tag="Bg")
                        if "nogather" in ablate:
                            nc.vector.memset(g[:, 0, :], 0.0)
                        else:
                            nc.gpsimd.dma_gather(
                                out_ap=g[:],
                                in_ap=bass.AP(vtb.tensor, vtb.offset + t * 2 * RPH * 64,
                                              [[64, 2 * RPH], [1, 128]]),
                                idxs_ap=Tw[:, t * 256:(t + 1) * 256], num_idxs=4096,
                                num_idxs_reg=4096, elem_size=128, elem_step=64,
                                single_packet=False)
                        if "nosamp" in ablate:
                            nc.vector.memset(samp[:, t * 64:(t + 1) * 64], 0.0)
                            continue
                        QB = 4096
                        sw = pSW.tile([128, QB], BF16, tag="Bsw")
                        nc.vector.tensor_tensor(
                            out=_ap(sw, 0, [[32, 128], [1, 32]]),
                            in0=_ap(g, 0, [[32, 128], [1, 32]]),
                            in1=_ap(w4, t * 128, [[1, 128], [0, 32]]),
                            op=A.mult)
                        # in-place pairwise tree over the 128 32-el blocks
                        for n in (64, 32, 16, 8, 4):
                            nc.vector.tensor_tensor(
                                out=_ap(sw, 0, [[32, n], [1, 32]]),
                                in0=_ap(sw, 0, [[64, n], [1, 32]]),
                                in1=_ap(sw, 32, [[64, n], [1, 32]]), op=A.add)
                        nc.vector.tensor_tensor(
                            out=samp[:, t * 64:(t + 1) * 64],
                            in0=_ap(sw, 0, [[64, 2], [1, 32]]),
                            in1=_ap(sw, 32, [[64, 2], [1, 32]]), op=A.add)

                    # output projection
                    sT = pFin.tile([128, 2, 128], F32, tag="BsT")
                    for k in range(2):
                        tp = psT.tile([128, 128], F32, tag="Btp")
                        nc.tensor.transpose(tp[:], samp[:, k * 128:(k + 1) * 128], tid[:])
                        nc.scalar.copy(sT[:, k, :], tp[:])
                    o2p = psO.tile([128, D], F32, tag="Bo23")
                    nc.tensor.matmul(o2p[:], lhsT=sT[:, 0, :], rhs=tWout[:, 0:D], start=True, stop=False)
                    nc.tensor.matmul(o2p[:], lhsT=sT[:, 1, :], rhs=tWout[:, D:2 * D], start=False, stop=False)
                    nc.tensor.matmul(o2p[:], lhsT=tones[:], rhs=tbout[:], start=False, stop=True)

                    # residual + layernorm
                    def layer_norm(inp_sbuf, res_psum, gt, bt, tagp):
                        x1 = pFin.tile([128, D], F32, tag="BLx1")
                        sums = pFin.tile([128, 1], F32, tag="BLsu")
                        nc.vector.scalar_tensor_tensor(
                            out=x1[:], in0=inp_sbuf[:], scalar=0.0, in1=res_psum[:],
                            op0=A.add, op1=A.add, accum_out=sums[:])
                        negm = pFin.tile([128, 1], F32, tag="BLnm")
                        nc.scalar.mul(negm[:], sums[:], -1.0 / D)
                        sq = pFin.tile([128, D], F32, tag="BLsq")
                        ssq = pFin.tile([128, 1], F32, tag="BLss")
                        nc.scalar.activation(sq[:], x1[:], ACTF.Square,
                                             bias=negm[:, 0:1], accum_out=ssq[:])
                        sd = pFin.tile([128, 1], F32, tag="BLsd")
                        nc.scalar.activation(sd[:], ssq[:], ACTF.Sqrt,
                                             scale=1.0 / D, bias=teps[:, 0:1])
                        rstd = pFin.tile([128, 1], F32, tag="BLrs")
                        nc.vector.reciprocal(rstd[:], sd[:])
                        xh = pFin.tile([128, D], F32, tag="BLxh")
                        nc.vector.tensor_scalar(
                            out=xh[:], in0=x1[:], scalar1=negm[:, 0:1],
                            scalar2=rstd[:, 0:1], op0=A.add, op1=A.mult)
                        yv = pFin.tile([128, D], F32, tag=tagp + "y")
                        nc.vector.tensor_tensor(out=yv[:], in0=xh[:], in1=gt[:], op=A.mult)
                        nc.vector.tensor_tensor(out=yv[:], in0=yv[:], in1=bt[:], op=A.add)
                        return yv

                    y1v = layer_norm(s, o2p, tg1, tbe1, "BL1")

                    # FFN
                    yT = pFin.tile([128, 2, 128], BF16, tag="ByT")
                    for k in range(2):
                        tp = psT.tile([128, 128], F32, tag="Btp")
                        nc.tensor.transpose(tp[:], y1v[:, k * 128:(k + 1) * 128], tid[:])
                        nc.scalar.copy(yT[:, k, :], tp[:])
                    h1 = pFin.tile([128, DFF], BF16, tag="Bh1")
                    for j in range(8):
                        js = slice(j * 128, (j + 1) * 128)
                        hp = psF.tile([128, 128], F32, tag="Bhp")
                        nc.tensor.matmul(hp[:], lhsT=tW1[:, 0 * DFF + j * 128:0 * DFF + (j + 1) * 128],
                                         rhs=yT[:, 0, :], start=True, stop=False)
                        nc.tensor.matmul(hp[:], lhsT=tW1[:, 1 * DFF + j * 128:1 * DFF + (j + 1) * 128],
                                         rhs=yT[:, 1, :], start=False, stop=False)
                        nc.tensor.matmul(hp[:], lhsT=tb1b[:, js], rhs=tonesb[:], start=False, stop=True)
                        nc.scalar.activation(h1[:, js], hp[:], ACTF.Relu)
                    o3p = psO.tile([128, D], F32, tag="Bo23")
                    for j in range(8):
                        js = slice(j * 128, (j + 1) * 128)
                        nc.tensor.matmul(o3p[:], lhsT=h1[:, js], rhs=tW2[:, j * D:(j + 1) * D],
                                         start=(j == 0), stop=False)
                    nc.tensor.matmul(o3p[:], lhsT=tonesb[:], rhs=tb2b[:], start=False, stop=True)

                    y2v = layer_norm(y1v, o3p, tg2, tbe2, "BL2")
                    nc.sync.dma_start(outq[rs, :], y2v[:])
                    if dbg:
                        nc.sync.dma_start(d_ofs[rs, :], d_offs_t[:])
                        nc.sync.dma_start(d_aw[rs, :], d_aw_t[:])
                        nc.sync.dma_start(d_w4[rs, :], w4[:])
                        nc.sync.dma_start(d_samp[rs, :], samp[:])
                        nc.sync.dma_start(d_x0[rs, :], d_x0_t[:])
                        nc.sync.dma_start(d_y0[rs, :], d_y0_t[:])

                # software pipeline: prefill PRE preps (they run during
                # Phase A), then issue sample-first so the DVE reaches each
                # tile's weighted-reduce immediately while preps stay 4
                # tiles ahead in every engine FIFO.
                ntiles = 0 if "nob" in ablate else N_Q_TILES
                PRE = 4
                states = [prep(i) for i in range(min(PRE, ntiles))]
                for i in range(ntiles):
                    sample(states[i])
                    j = i + PRE
                    if j < ntiles:
                        states.append(prep(j))

    nc.compile()
    return nc


# ----------------------------------------------------------------------
# host-side wrapper
# ----------------------------------------------------------------------
_NC_CACHE = None


def _get_nc():
    global _NC_CACHE
    if _NC_CACHE is None:
        _NC_CACHE = build()
    return _NC_CACHE


def make_consts():
    h_i, l_i, p_i = np.meshgrid(np.arange(NH), np.arange(NL), np.arange(NP), indexing="ij")
    Wl = np.array([w for (_, w) in SPATIAL], np.float32)
    Hl = np.array([h for (h, _) in SPATIAL], np.float32)
    lw = Wl[l_i].reshape(-1)
    lh = Hl[l_i].reshape(-1)
    rspc = np.array(RSPC, np.float32)[l_i].reshape(-1)
    lrb = np.array(LRB, np.float32)[l_i].reshape(-1)
    b2 = ((h_i % 2) * RPH).reshape(-1) + lrb + rspc + 1.0
    rep = lambda v: np.tile(v[None, :].astype(np.float32), (128, 1))
    dims8 = np.zeros(NL * 2, np.float32)
    dims8[0::2] = Wl
    dims8[1::2] = Hl
    Sall = np.zeros((128, 8 * 128), np.float32)
    for qh in range(8):
        for q16 in range(16):
            for k in range(8):
                Sall[16 * qh + q16, qh * 128 + 16 * k + q16] = 1.0
    return {
        "cW": rep(lw), "cH": rep(lh),
        "cWm1": rep(lw - 1), "cHm1": rep(lh - 1),
        "cWm2": rep(lw - 2), "cHm2": rep(lh - 2),
        "cRSPC": rep(rspc), "cB2": rep(b2),
        "dims8": rep(dims8),
        "ident": np.eye(128, dtype=np.float32),
        "ones_row": np.ones((1, 128), np.float32),
        "Sall": Sall,
        "zeros8k": np.zeros((1, 8192), np.float32),
    }


SHARD_STARTS = [0, 3324, 6648, 9972]
SHARD_SIZES = [3324, 3324, 3324, 3322]


def make_in_maps(inputs):
    consts = make_consts()
    in_maps = []
    for core in range(8):
        b, c = core // 4, core % 4
        st, sz = SHARD_STARTS[c], SHARD_SIZES[c]
        src_full = np.zeros((PAD_LEN, D), np.float32)
        src_full[:LEN] = inputs["src"][b]
        srcq = np.zeros((Q_SH, D), np.float32)
        srcq[:sz] = inputs["src"][b, st:st + sz]
        posq = np.zeros((Q_SH, D), np.float32)
        posq[:sz] = inputs["pos"][b, st:st + sz]
        refq = np.full((Q_SH, NL * 2), 0.5, np.float32)
        refq[:sz] = inputs["reference_points"][b, st:st + sz].reshape(sz, NL * 2)
        m = {
            "src_full": src_full, "srcq": srcq, "posq": posq, "refq": refq,
            "Wv": inputs["W_value"], "Woff": inputs["W_off"],
            "Wattn": inputs["W_attn"], "Wout": inputs["W_out"],
            "W1": inputs["W1"], "W2": inputs["W2"],
            "bv": inputs["b_value"][None, :], "boff": inputs["b_off"][None, :],
            "battn": inputs["b_attn"][None, :], "bout": inputs["b_out"][None, :],
            "b1": inputs["b1"][None, :], "b2": inputs["b2"][None, :],
            "g1r": np.tile(inputs["g1"][None, :], (128, 1)),
            "be1r": np.tile(inputs["be1"][None, :], (128, 1)),
            "g2r": np.tile(inputs["g2"][None, :], (128, 1)),
            "be2r": np.tile(inputs["be2"][None, :], (128, 1)),
        }
        for k in ("cW", "cH", "cWm1", "cHm1", "cWm2", "cHm2", "cRSPC", "cB2",
                  "dims8", "ident", "ones_row", "Sall", "zeros8k"):
            m[k] = consts[k]
        import ml_dtypes
        bf16_params = {"Wv", "W1", "W2"}
        in_maps.append({
            k: np.ascontiguousarray(v, ml_dtypes.bfloat16 if k in bf16_params else np.float32)
            for k, v in m.items()})
    return in_maps


def assemble_out(results):
    out = np.empty((2, LEN, D), np.float32)
    for core in range(8):
        b, c = core // 4, core % 4
        st, sz = SHARD_STARTS[c], SHARD_SIZES[c]
        out[b, st:st + sz] = results[core]["outq"][:sz]
    return out


def run(inputs, trace=False, **kw):
    nc = _get_nc()
    in_maps = make_in_maps(inputs)
    res = run_bass_kernel_spmd(nc, in_maps, core_ids=list(range(8)), trace=trace, **kw)
    return assemble_out(res.results), res


def kernel(**inputs):
    out, _ = run(inputs)
    return out
